# revision 51
# baseline (speedup 1.0000x reference)
"""Differential attention (two-softmax diff + GroupNorm) on 8 TRN2 cores.

Sharding: 16 heads / 8 cores = 2 heads per core (head-parallel, no
collectives). GroupNorm stats are per-(batch, head) so each core is fully
independent.

Device layout (host prepares everything):
  - Q, K per head are host-transposed to [128(d), 2048(s)] fp16: partitions
    0-63 hold half-1 (q1/k1), partitions 64-127 hold half-2. QK^T contracts
    over the partition dim, producing transposed score blocks S^T[key, query]
    in PSUM (fp32). The two 64-contraction halves auto-derive PE row-tile
    positions (0,0)/(64,0) and execute concurrently on the array.
  - V per head is prefixed with a ones column (V' = [1 | V], 65 cols, fp16)
    pre-arranged as [128(key-in-block), 16*65]: the PV matmul yields the
    softmax denominator on partition 0 and the numerator on partitions 1-64.
    lam is folded into half-2's V on the host.

ScalarE's exp over 2*S^2 scores/head is the bottleneck engine (~1.1 ns per
lane-element + ~260ns fixed per ACTIVATE). Levers:
  1. Bigger activation batches: scores accumulate in two rotating 3-bank
     PSUM tiles [128, 1536] so each ACTIVATE covers 3 slabs.
  2. ~25% of slab-groups are exp'd on the VectorE via a 2-instruction
     Schraudolph pipeline with cubic mantissa correction (max rel err
     ~7.7e-4, same class as the fp16 store quantization):
       i   = round_f32(s*A2 + B2)      stock tensor_scalar, f32->i32 convert
       w   = bitcast_f32(i)            = 2^(t+delta) * (1+f)/2^f
       m   = (i & 0x7FFFFF) | One.bits custom op: mantissa -> [1,2)
       e   = (((p'-m)m + q')m + r') * w   cubic corr, |c3| folded into B2
     The finisher is one 8-stage custom-DVE op (and, or, sub, mul, add,
     mul, add, mul) registered at import time.

Scheduling: per chunk, pass A emits all QK + exp (e production runs ahead),
pass B the serial PV accumulation; the next chunk's first two score groups
are emitted before this chunk's tail PVs (software-pipelined boundary).
Accumulators are evicted by ScalarE copies slotted into the next chunk's
act stream; the normalize (reciprocal_approx_fast on the den rows, GpSimd
partition broadcasts, multiply/subtract) is deferred behind the next
chunk's exp work on the DVE. bn stats stay on-device per chunk; the final
64-way scalar stat reduction, rsqrt and GroupNorm affine are applied on
the host during unsharding (outT carries the un-affined diff, sgtb the
per-partition (mean, var)).
"""

import math

import numpy as np

B, H, S, D = 1, 16, 2048, 64
N_CORES = 8
HPC = H // N_CORES  # heads per core
QC = 512            # query-chunk width
N_QC = S // QC
KB = S // 128       # key blocks of 128
LAMBDA_INIT = 0.8
EPS = 1e-5
SCALE = 1.0 / math.sqrt(D)
N_WARMUP_MM = 14

# cubic minimax fit of R(m) = 2^(m-1)/m on [1,2):  c3 m^3 + c2 m^2 + c1 m + c0
_C3 = -0.10246085749846692
_C2 = 0.69063801
_C1 = -1.35417106
_C0 = 1.76527539
PPRIME = -_C2 / _C3            # +6.7405058
QPRIME = -_C1 / _C3            # -13.216472
RPRIME = -_C0 / _C3            # +17.228778
DELTA = math.log2(-_C3)        # fold |c3| into the exponent bias
A2 = float(np.float32(math.log2(math.e) * SCALE * 2.0**23))
B2 = float(np.float32((127.0 + DELTA) * 2.0**23))

# slab-groups per 512-query chunk: 32 slabs of [128,512] scores -> 11 groups
GROUPS = [(i * 3, 3) for i in range(10)] + [(30, 2)]
# group indices handled by the VectorE exp pipeline (rest: ScalarE ACTIVATE);
# mid placement keeps the e-latency off both the rotation head and the PV
# chain tail; alternation balances the two engines at ~2.5 groups/chunk
DVE_GROUPS_EVEN = (2, 5, 8)
DVE_GROUPS_ODD = (2, 5, 8)
DVE_GROUPS_LAST = (3, 7)

_CACHE = {}


def _get_exp_op():
    """Register (once) and return the custom-DVE exp-finisher op."""
    if "expop" in _CACHE:
        return _CACHE["expop"]
    from concourse import dve_ops
    from concourse.dve_spec import (
        AluOp,
        Bin,
        C0,
        C1,
        C2,
        C3,
        One,
        Spec,
        Src0,
        _spill_c3_to_src1,
        lower,
    )
    from concourse.dve_uop import DveOpSpec

    for existing in dve_ops.OPS:
        if existing.name == "ANT_EXP2_FINISH":
            _CACHE["expop"] = existing
            return existing

    mm = Bin(AluOp.BITWISE_AND, Src0, C0)
    mo = Bin(AluOp.BITWISE_OR, mm, One)
    t5 = ((C1 - mo) * mo + C2) * mo + C3
    body = _spill_c3_to_src1(t5 * Src0)

    def _ref(in0, in1, s0, s1, imm2):
        bits = np.asarray(in0, np.float32).view(np.int32)
        s0i = np.asarray(s0).view(np.int32) if isinstance(s0, np.ndarray) else np.int32(s0)
        m = ((bits & s0i) | np.int32(0x3F800000)).view(np.float32)
        t = ((np.float32(s1) - m) * m + np.float32(imm2)) * m + np.asarray(
            in1, np.float32
        )
        return t * np.asarray(in0, np.float32)

    spec = Spec(body=body, reference=_ref)
    op = dve_ops.DveOp("ANT_EXP2_FINISH", spec, subdim=False, uops_sha={})
    dve_ops.OPS.append(op)
    dve_ops._SUB_OPCODE_FOR_NAME[op.name] = dve_ops._CUSTOM_DVE_ROW_BASE + len(
        dve_ops.OPS
    ) - 1
    dve_ops.CUSTOM_DVE_SPECS[op.name] = spec
    for ver in ("v3", "v4"):
        tmp = DveOpSpec(
            name=op.name,
            opcode=dve_ops.get_dve_sub_opcode(op.name),
            uops=lower(spec, ver=ver),
            rd1_en=True,
        )
        op.uops_sha[ver] = tmp.sha(ver)
    _CACHE["expop"] = op
    return op


def _build_nc():
    from contextlib import ExitStack

    import concourse.bacc as bacc
    import concourse.bass as bass
    import concourse.tile as tile
    from concourse import bass_isa, mybir

    f32 = mybir.dt.float32
    f16 = mybir.dt.float16
    i32 = mybir.dt.int32
    AF = mybir.ActivationFunctionType
    OP = mybir.AluOpType
    ts = bass.ts

    expop = _get_exp_op()

    nc = bacc.Bacc("TRN2", target_bir_lowering=False, debug=False)

    qT = nc.dram_tensor("qT", [HPC, 128, S], f16, kind="ExternalInput").ap()
    kT = nc.dram_tensor("kT", [HPC, 128, S], f16, kind="ExternalInput").ap()
    vp = nc.dram_tensor("vp", [HPC, 2, 128, KB * 65], f16, kind="ExternalInput").ap()
    gb = nc.dram_tensor("gb", [HPC, 64, 2], f32, kind="ExternalInput").ap()
    outT = nc.dram_tensor("outT", [HPC, 64, S], f32, kind="ExternalOutput").ap()
    sgtb = nc.dram_tensor("sgtb", [HPC, 65, 2], f32, kind="ExternalOutput").ap()

    with tile.TileContext(nc) as tc, ExitStack() as ctx:
        pq = ctx.enter_context(tc.tile_pool(name="pq", bufs=2))
        pk = ctx.enter_context(tc.tile_pool(name="pk", bufs=2))
        pv = ctx.enter_context(tc.tile_pool(name="pv", bufs=2))
        pe = ctx.enter_context(tc.tile_pool(name="pe", bufs=13))
        pw = ctx.enter_context(tc.tile_pool(name="pw", bufs=1))
        pep = ctx.enter_context(tc.tile_pool(name="pep", bufs=2))
        psa = ctx.enter_context(tc.tile_pool(name="psa", bufs=2))
        pout = ctx.enter_context(tc.tile_pool(name="pout", bufs=2))
        pst = ctx.enter_context(tc.tile_pool(name="pst", bufs=2))
        psingle = ctx.enter_context(tc.tile_pool(name="psingle", bufs=1))
        psc = ctx.enter_context(tc.tile_pool(name="psc", bufs=1, space="PSUM"))
        pacc = ctx.enter_context(tc.tile_pool(name="pacc", bufs=1, space="PSUM"))

        def emit_loads(h):
            """DMA the head's inputs; split so the first matmuls start early."""
            ksh = []
            for j in range(2):
                ks_t = pk.tile([128, S // 2], f16, tag=f"ks{j}", name=f"ks{j}")
                ksh.append(ks_t)
            qsh = []
            for j in range(N_QC):
                qs_t = pq.tile([128, QC], f16, tag=f"qs{j}", name=f"qs{j}")
                qsh.append(qs_t)
            nc.sync.dma_start(ksh[0][:], kT[h, :, 0 : S // 2])
            nc.sync.dma_start(qsh[0][:], qT[h, :, 0:QC])
            nc.sync.dma_start(ksh[1][:], kT[h, :, S // 2 : S])
            for j in range(1, N_QC):
                nc.sync.dma_start(qsh[j][:], qT[h, :, j * QC : (j + 1) * QC])
            vsh = []
            for half in range(2):
                row = []
                for j in range(2):
                    t = pv.tile(
                        [128, KB * 65 // 2], f16, tag=f"v{half}{j}", name=f"v{half}{j}"
                    )
                    nc.sync.dma_start(
                        t[:],
                        vp[h, half, :, j * (KB * 65 // 2) : (j + 1) * (KB * 65 // 2)],
                    )
                    row.append(t)
                vsh.append(row)
            return ksh, qsh, vsh

        # PE warm-up: tiny back-to-back matmuls flip the HAM clock gate to
        # 8/8 while the first head's DMAs are in flight.
        wu_w = psingle.tile([128, 128], f16)
        nc.vector.memset(wu_w, 0.0)
        wu_ps = psc.tile([128, 3 * QC], f32, tag="sc0")
        for _ in range(N_WARMUP_MM):
            nc.tensor.matmul(
                wu_ps[:, 0:128], lhsT=wu_w[:], rhs=wu_w[:], start=True, stop=True
            )

        loads = emit_loads(0)

        mask_t = psingle.tile([128, 1], i32)
        nc.vector.memset(mask_t, 0x007FFFFF)
        rprime_t = psingle.tile([128, 1], f32)
        nc.vector.memset(rprime_t, RPRIME)

        for h in range(HPC):
            ksh, qsh, vsh = loads
            if h + 1 < HPC:
                # prefetch the next head's inputs behind this head's compute
                loads = emit_loads(h + 1)


            outc = pout.tile([65, S], f32)
            st = pst.tile([65, N_QC, 6], f32)

            e_tiles = {}  # (qc, g) -> e tile
            accs = {}     # qc -> (a1, a2)

            def emit_A(qc, g):
                """Scores + exp for group g of chunk qc."""
                gs, gl = GROUPS[g]
                L = gl * QC
                last = h == HPC - 1 and qc == N_QC - 1
                dve_groups = (
                    DVE_GROUPS_LAST
                    if last
                    else (DVE_GROUPS_EVEN if qc % 2 == 0 else DVE_GROUPS_ODD)
                )
                sc = psc.tile([128, 3 * QC], f32, tag=f"sc{g % 2}")
                for i in range(gs, gs + gl):
                    kb, half = divmod(i, 2)
                    col = (i - gs) * QC
                    ksk = ksh[kb // 8][:, ts(kb % 8, 128)]
                    nc.tensor.matmul(
                        sc[:, col : col + QC],
                        lhsT=ksk[64 * half : 64 * (half + 1), :],
                        rhs=qsh[qc][64 * half : 64 * (half + 1), :],
                        start=True,
                        stop=True,
                    )
                e = pe.tile([128, 3 * QC], f16, tag="e")
                e_tiles[(qc, g)] = e
                if g in dve_groups:
                    w32 = pw.tile([128, 3 * QC], i32, tag=f"w{g % 2}")
                    nc.vector.tensor_scalar(
                        out=w32[:, 0:L],
                        in0=sc[:, 0:L],
                        scalar1=A2,
                        scalar2=B2,
                        op0=OP.mult,
                        op1=OP.add,
                    )
                    nc.vector._custom_dve(
                        expop,
                        out=e[:, 0:L],
                        in0=w32[:, 0:L].bitcast(f32),
                        in1=rprime_t[:],
                        s0=mask_t[:].bitcast(f32),
                        s1=PPRIME,
                        imm2=QPRIME,
                    )
                else:
                    nc.scalar.activation(e[:, 0:L], sc[:, 0:L], AF.Exp, scale=SCALE)

            def emit_B(qc, g):
                """PV accumulation for group g of chunk qc."""
                gs, gl = GROUPS[g]
                a1, a2 = accs[qc]
                for i in range(gs, gs + gl):
                    kb, half = divmod(i, 2)
                    col = (i - gs) * QC
                    acc = a1 if half == 0 else a2
                    vk = vsh[half][kb // 8][:, ts(kb % 8, 65)]
                    nc.tensor.matmul(
                        acc[:],
                        lhsT=vk,
                        rhs=e_tiles.pop((qc, g))[:, col : col + QC]
                        if i == gs + gl - 1
                        else e_tiles[(qc, g)][:, col : col + QC],
                        start=(kb == 0),
                        stop=(kb == KB - 1),
                    )

            pending_evict = []
            pending_pre = []
            pending = []
            for qc in range(N_QC):
                last = h == HPC - 1 and qc == N_QC - 1
                if qc == 0:
                    emit_A(0, 0)
                    emit_A(0, 1)
                for g in range(2, len(GROUPS)):
                    if g == 2:
                        # previous chunk's accumulator evictions slot in here
                        # so they never head-of-line block this chunk's first
                        # activations on the ScalarE queue; the reciprocals
                        # follow and fill the DVE's early idle window
                        for fn in pending_evict:
                            fn()
                        pending_evict = []
                        for fn in pending_pre:
                            fn()
                        pending_pre = []
                    emit_A(qc, g)
                # previous chunk's deferred combine sits behind this chunk's
                # exp stream on the DVE
                for fn in pending:
                    fn()
                pending = []

                acc1 = pacc.tile([65, QC], f32, tag="a1")
                acc2 = pacc.tile([65, QC], f32, tag="a2")
                accs[qc] = (acc1, acc2)
                for g in range(0, 9):
                    emit_B(qc, g)
                # software-pipeline the chunk boundary: the next chunk's first
                # score groups go ahead of this chunk's tail PVs in the PE
                # stream so the activation rotation never stalls on them
                if qc + 1 < N_QC:
                    emit_A(qc + 1, 0)
                    emit_A(qc + 1, 1)
                for g in range(9, len(GROUPS)):
                    emit_B(qc, g)

                a1, a2 = accs.pop(qc)
                if last:
                    # banks are free after this — normalize straight out of
                    # PSUM, halves pipelined, to shorten the exposed tail
                    rb1r = pst.tile([1, QC], f32, tag="rb1r")
                    nc.vector.reciprocal_approx_fast(rb1r[:], a1[0:1, :])
                    rb1 = pep.tile([65, QC], f32, tag="rb1")
                    nc.gpsimd.partition_broadcast(rb1[:], rb1r[:], channels=65)
                    rb2r = pst.tile([1, QC], f32, tag="rb2r")
                    nc.vector.reciprocal_approx_fast(rb2r[:], a2[0:1, :])
                    rb2 = pep.tile([65, QC], f32, tag="rb2")
                    nc.gpsimd.partition_broadcast(rb2[:], rb2r[:], channels=65)
                    t1 = pep.tile([65, QC], f32, tag="lt1")
                    nc.vector.tensor_mul(t1[:], a1[:], rb1[:])
                    t2 = pep.tile([65, QC], f32, tag="lt2")
                    nc.vector.tensor_mul(t2[:], a2[:], rb2[:])
                    nc.vector.tensor_sub(outc[:, ts(qc, QC)], t1[:], t2[:])
                    nc.vector.bn_stats(st[:, qc, :], outc[:, ts(qc, QC)])
                    nc.sync.dma_start(outT[h, :, ts(qc, QC)], outc[1:65, ts(qc, QC)])
                else:
                    # both halves side by side in one staging tile so the
                    # deferred normalize is one wide op per stage
                    sa = psa.tile([65, 2 * QC], f32, tag=f"sa_{qc % 2}")

                    def _evict(a1=a1, a2=a2, sa=sa):
                        nc.scalar.copy(sa[:, 0:QC], a1[:])
                        nc.scalar.copy(sa[:, QC:], a2[:])

                    def _recips(sa=sa):
                        rbr = pst.tile([1, 2 * QC], f32, tag="rbr")
                        nc.vector.reciprocal_approx_fast(rbr[:], sa[0:1, :])
                        rb = pep.tile([65, 2 * QC], f32, tag="rb")
                        nc.gpsimd.partition_broadcast(rb[:], rbr[:], channels=65)
                        return rb

                    def _combine(rb, qc=qc, sa=sa, h=h):
                        tt = pep.tile([65, 2 * QC], f32, tag=f"tt_{qc % 2}")
                        nc.vector.tensor_mul(tt[:], sa[:], rb[:])
                        nc.vector.tensor_sub(
                            outc[:, ts(qc, QC)], tt[:, 0:QC], tt[:, QC:]
                        )
                        nc.vector.bn_stats(st[:, qc, :], outc[:, ts(qc, QC)])
                        # un-affined diff streams out now; the host applies
                        # the per-head GroupNorm affine during unsharding
                        nc.sync.dma_start(
                            outT[h, :, ts(qc, QC)], outc[1:65, ts(qc, QC)]
                        )

                    pending_evict.append(_evict)
                    rbs = []
                    pending_pre.append(lambda r=rbs, f=_recips: r.append(f()))
                    pending.append(lambda r=rbs, f=_combine: f(r[0]))
            for fn in pending_evict:
                fn()
            pending_evict = []
            for fn in pending_pre:
                fn()
            pending_pre = []
            for fn in pending:
                fn()
            pending = []

            # ---- head finalize: per-partition (mean, var) over S leave the
            # device; the 64-way scalar reduction + rsqrt + affine happen on
            # the host during unsharding ----
            mv = pst.tile([65, 2], f32)
            nc.vector.bn_aggr(mv[:], st[:])
            nc.sync.dma_start(sgtb[h], mv[:])

    nc.compile()
    return nc


def _get_nc():
    if "nc" not in _CACHE:
        _CACHE["nc"] = _build_nc()
    return _CACHE["nc"]


def _host_prep(q, k, v, lq1, lq2, lk1, lk2, gamma, beta):
    """Build per-core input maps."""
    q = np.asarray(q, dtype=np.float32)
    k = np.asarray(k, dtype=np.float32)
    v = np.asarray(v, dtype=np.float32)
    lam = float(
        np.exp(np.float32(np.dot(lq1, lk1)))
        - np.exp(np.float32(np.dot(lq2, lk2)))
        + LAMBDA_INIT
    )
    g2 = (np.asarray(gamma, np.float32) * (1.0 - LAMBDA_INIT)).reshape(H, D)
    b2 = (np.asarray(beta, np.float32) * (1.0 - LAMBDA_INIT)).reshape(H, D)

    in_maps = []
    for c in range(N_CORES):
        heads = range(c * HPC, (c + 1) * HPC)
        qTa = np.empty((HPC, 128, S), np.float16)
        kTa = np.empty((HPC, 128, S), np.float16)
        vpa = np.empty((HPC, 2, 128, KB * 65), np.float16)
        gba = np.empty((HPC, 64, 2), np.float32)
        for i, hh in enumerate(heads):
            qTa[i] = q[0, hh].T.astype(np.float16)
            kTa[i] = k[0, hh].T.astype(np.float16)
            vh = v[0, hh]  # [S, 64]
            v1 = np.concatenate([np.ones((S, 1), np.float32), vh], axis=1)
            v2 = np.concatenate([np.ones((S, 1), np.float32), lam * vh], axis=1)
            # SBUF image: [partition(key within block), kblock*65 + col]
            vpa[i, 0] = (
                v1.reshape(KB, 128, 65).transpose(1, 0, 2).reshape(128, KB * 65)
            ).astype(np.float16)
            vpa[i, 1] = (
                v2.reshape(KB, 128, 65).transpose(1, 0, 2).reshape(128, KB * 65)
            ).astype(np.float16)
            gba[i, :, 0] = g2[hh]
            gba[i, :, 1] = b2[hh]
        in_maps.append({"qT": qTa, "kT": kTa, "vp": vpa, "gb": gba})
    return in_maps


def kernel(q, k, v, lq1, lq2, lk1, lk2, gamma, beta, _trace=False, _tmpdir=None):
    from concourse.bass_utils import run_bass_kernel_spmd

    nc = _get_nc()
    in_maps = _host_prep(q, k, v, lq1, lq2, lk1, lk2, gamma, beta)
    res = run_bass_kernel_spmd(
        nc,
        in_maps,
        core_ids=list(range(N_CORES)),
        trace=_trace,
        tmpdir=_tmpdir,
    )
    g2 = (np.asarray(gamma, np.float32) * (1.0 - LAMBDA_INIT)).reshape(H, D)
    b2 = (np.asarray(beta, np.float32) * (1.0 - LAMBDA_INIT)).reshape(H, D)
    out = np.empty((B, H, S, D), np.float32)
    for c in range(N_CORES):
        outT = res.results[c]["outT"]  # [HPC, 64, S] un-affined diff
        mvs = res.results[c]["sgtb"]   # [HPC, 65, 2] per-partition (mean, var)
        for i in range(HPC):
            hh = c * HPC + i
            mean_p = mvs[i, 1:65, 0].astype(np.float64)
            var_p = mvs[i, 1:65, 1].astype(np.float64)
            mu = mean_p.mean()
            var = (var_p + mean_p**2).mean() - mu * mu
            rstd = 1.0 / np.sqrt(var + EPS)
            sg = (rstd * g2[hh]).astype(np.float32)[:, None]
            tb = (b2[hh] - mu * rstd * g2[hh]).astype(np.float32)[:, None]
            out[0, hh] = (outT[i] * sg + tb).T
    if _trace:
        _CACHE["last_results"] = res
    return out


# revision 52
# speedup vs baseline: 1.0083x; 1.0083x over previous
"""Differential attention (two-softmax diff + GroupNorm) on 8 TRN2 cores.

Sharding: 16 heads / 8 cores = 2 heads per core (head-parallel, no
collectives). GroupNorm stats are per-(batch, head) so each core is fully
independent.

Device layout (host prepares everything):
  - Q, K per head are host-transposed to [128(d), 2048(s)] fp16: partitions
    0-63 hold half-1 (q1/k1), partitions 64-127 hold half-2. QK^T contracts
    over the partition dim, producing transposed score blocks S^T[key, query]
    in PSUM (fp32). The two 64-contraction halves auto-derive PE row-tile
    positions (0,0)/(64,0) and execute concurrently on the array.
  - V per head is prefixed with a ones column (V' = [1 | V], 65 cols, fp16)
    pre-arranged as [128(key-in-block), 16*65]: the PV matmul yields the
    softmax denominator on partition 0 and the numerator on partitions 1-64.
    lam is folded into half-2's V on the host.

ScalarE's exp over 2*S^2 scores/head is the bottleneck engine (~1.1 ns per
lane-element + ~260ns fixed per ACTIVATE). Levers:
  1. Bigger activation batches: scores accumulate in two rotating 3-bank
     PSUM tiles [128, 1536] so each ACTIVATE covers 3 slabs.
  2. ~25% of slab-groups are exp'd on the VectorE via a 2-instruction
     Schraudolph pipeline with cubic mantissa correction (max rel err
     ~7.7e-4, same class as the fp16 store quantization):
       i   = round_f32(s*A2 + B2)      stock tensor_scalar, f32->i32 convert
       w   = bitcast_f32(i)            = 2^(t+delta) * (1+f)/2^f
       m   = (i & 0x7FFFFF) | One.bits custom op: mantissa -> [1,2)
       e   = (((p'-m)m + q')m + r') * w   cubic corr, |c3| folded into B2
     The finisher is one 8-stage custom-DVE op (and, or, sub, mul, add,
     mul, add, mul) registered at import time.

Scheduling: per chunk, pass A emits all QK + exp (e production runs ahead),
pass B the serial PV accumulation; the next chunk's first two score groups
are emitted before this chunk's tail PVs (software-pipelined boundary).
Accumulators are evicted by ScalarE copies slotted into the next chunk's
act stream; the normalize (reciprocal_approx_fast on the den rows, GpSimd
partition broadcasts, multiply/subtract) is deferred behind the next
chunk's exp work on the DVE. bn stats stay on-device per chunk; the final
64-way scalar stat reduction, rsqrt and GroupNorm affine are applied on
the host during unsharding (outT carries the un-affined diff, sgtb the
per-partition (mean, var)).
"""

import math

import numpy as np

B, H, S, D = 1, 16, 2048, 64
N_CORES = 8
HPC = H // N_CORES  # heads per core
QC = 512            # query-chunk width
N_QC = S // QC
KB = S // 128       # key blocks of 128
LAMBDA_INIT = 0.8
EPS = 1e-5
SCALE = 1.0 / math.sqrt(D)
N_WARMUP_MM = 14

# cubic minimax fit of R(m) = 2^(m-1)/m on [1,2):  c3 m^3 + c2 m^2 + c1 m + c0
_C3 = -0.10246085749846692
_C2 = 0.69063801
_C1 = -1.35417106
_C0 = 1.76527539
PPRIME = -_C2 / _C3            # +6.7405058
QPRIME = -_C1 / _C3            # -13.216472
RPRIME = -_C0 / _C3            # +17.228778
DELTA = math.log2(-_C3)        # fold |c3| into the exponent bias
A2 = float(np.float32(math.log2(math.e) * SCALE * 2.0**23))
B2 = float(np.float32((127.0 + DELTA) * 2.0**23))

# slab-groups per 512-query chunk: 32 slabs of [128,512] scores -> 11 groups
GROUPS = [(i * 3, 3) for i in range(10)] + [(30, 2)]
# group indices handled by the VectorE exp pipeline (rest: ScalarE ACTIVATE);
# mid placement keeps the e-latency off both the rotation head and the PV
# chain tail; alternation balances the two engines at ~2.5 groups/chunk
DVE_GROUPS_EVEN = (2, 5, 8)
DVE_GROUPS_ODD = (3, 7)
DVE_GROUPS_LAST = (3, 7)

_CACHE = {}


def _get_exp_op():
    """Register (once) and return the custom-DVE exp-finisher op."""
    if "expop" in _CACHE:
        return _CACHE["expop"]
    from concourse import dve_ops
    from concourse.dve_spec import (
        AluOp,
        Bin,
        C0,
        C1,
        C2,
        C3,
        One,
        Spec,
        Src0,
        _spill_c3_to_src1,
        lower,
    )
    from concourse.dve_uop import DveOpSpec

    for existing in dve_ops.OPS:
        if existing.name == "ANT_EXP2_FINISH":
            _CACHE["expop"] = existing
            return existing

    mm = Bin(AluOp.BITWISE_AND, Src0, C0)
    mo = Bin(AluOp.BITWISE_OR, mm, One)
    t5 = ((C1 - mo) * mo + C2) * mo + C3
    body = _spill_c3_to_src1(t5 * Src0)

    def _ref(in0, in1, s0, s1, imm2):
        bits = np.asarray(in0, np.float32).view(np.int32)
        s0i = np.asarray(s0).view(np.int32) if isinstance(s0, np.ndarray) else np.int32(s0)
        m = ((bits & s0i) | np.int32(0x3F800000)).view(np.float32)
        t = ((np.float32(s1) - m) * m + np.float32(imm2)) * m + np.asarray(
            in1, np.float32
        )
        return t * np.asarray(in0, np.float32)

    spec = Spec(body=body, reference=_ref)
    op = dve_ops.DveOp("ANT_EXP2_FINISH", spec, subdim=False, uops_sha={})
    dve_ops.OPS.append(op)
    dve_ops._SUB_OPCODE_FOR_NAME[op.name] = dve_ops._CUSTOM_DVE_ROW_BASE + len(
        dve_ops.OPS
    ) - 1
    dve_ops.CUSTOM_DVE_SPECS[op.name] = spec
    for ver in ("v3", "v4"):
        tmp = DveOpSpec(
            name=op.name,
            opcode=dve_ops.get_dve_sub_opcode(op.name),
            uops=lower(spec, ver=ver),
            rd1_en=True,
        )
        op.uops_sha[ver] = tmp.sha(ver)
    _CACHE["expop"] = op
    return op


def _build_nc():
    from contextlib import ExitStack

    import concourse.bacc as bacc
    import concourse.bass as bass
    import concourse.tile as tile
    from concourse import bass_isa, mybir

    f32 = mybir.dt.float32
    f16 = mybir.dt.float16
    i32 = mybir.dt.int32
    AF = mybir.ActivationFunctionType
    OP = mybir.AluOpType
    ts = bass.ts

    expop = _get_exp_op()

    nc = bacc.Bacc("TRN2", target_bir_lowering=False, debug=False)

    qT = nc.dram_tensor("qT", [HPC, 128, S], f16, kind="ExternalInput").ap()
    kT = nc.dram_tensor("kT", [HPC, 128, S], f16, kind="ExternalInput").ap()
    vp = nc.dram_tensor("vp", [HPC, 2, 128, KB * 65], f16, kind="ExternalInput").ap()
    gb = nc.dram_tensor("gb", [HPC, 64, 2], f32, kind="ExternalInput").ap()
    outT = nc.dram_tensor("outT", [HPC, 64, S], f32, kind="ExternalOutput").ap()
    sgtb = nc.dram_tensor("sgtb", [HPC, 65, 2], f32, kind="ExternalOutput").ap()

    with tile.TileContext(nc) as tc, ExitStack() as ctx:
        pq = ctx.enter_context(tc.tile_pool(name="pq", bufs=2))
        pk = ctx.enter_context(tc.tile_pool(name="pk", bufs=2))
        pv = ctx.enter_context(tc.tile_pool(name="pv", bufs=2))
        pe = ctx.enter_context(tc.tile_pool(name="pe", bufs=13))
        pw = ctx.enter_context(tc.tile_pool(name="pw", bufs=1))
        pep = ctx.enter_context(tc.tile_pool(name="pep", bufs=2))
        psa = ctx.enter_context(tc.tile_pool(name="psa", bufs=2))
        pout = ctx.enter_context(tc.tile_pool(name="pout", bufs=2))
        pst = ctx.enter_context(tc.tile_pool(name="pst", bufs=2))
        psingle = ctx.enter_context(tc.tile_pool(name="psingle", bufs=1))
        psc = ctx.enter_context(tc.tile_pool(name="psc", bufs=1, space="PSUM"))
        pacc = ctx.enter_context(tc.tile_pool(name="pacc", bufs=1, space="PSUM"))

        def emit_loads(h):
            """DMA the head's inputs; split so the first matmuls start early."""
            ksh = []
            for j in range(2):
                ks_t = pk.tile([128, S // 2], f16, tag=f"ks{j}", name=f"ks{j}")
                ksh.append(ks_t)
            qsh = []
            for j in range(N_QC):
                qs_t = pq.tile([128, QC], f16, tag=f"qs{j}", name=f"qs{j}")
                qsh.append(qs_t)
            nc.sync.dma_start(ksh[0][:], kT[h, :, 0 : S // 2])
            nc.sync.dma_start(qsh[0][:], qT[h, :, 0:QC])
            nc.sync.dma_start(ksh[1][:], kT[h, :, S // 2 : S])
            for j in range(1, N_QC):
                nc.sync.dma_start(qsh[j][:], qT[h, :, j * QC : (j + 1) * QC])
            vsh = []
            for half in range(2):
                row = []
                for j in range(2):
                    t = pv.tile(
                        [128, KB * 65 // 2], f16, tag=f"v{half}{j}", name=f"v{half}{j}"
                    )
                    nc.sync.dma_start(
                        t[:],
                        vp[h, half, :, j * (KB * 65 // 2) : (j + 1) * (KB * 65 // 2)],
                    )
                    row.append(t)
                vsh.append(row)
            return ksh, qsh, vsh

        # PE warm-up: tiny back-to-back matmuls flip the HAM clock gate to
        # 8/8 while the first head's DMAs are in flight.
        wu_w = psingle.tile([128, 128], f16)
        nc.vector.memset(wu_w, 0.0)
        wu_ps = psc.tile([128, 3 * QC], f32, tag="sc0")
        for _ in range(N_WARMUP_MM):
            nc.tensor.matmul(
                wu_ps[:, 0:128], lhsT=wu_w[:], rhs=wu_w[:], start=True, stop=True
            )

        loads = emit_loads(0)

        mask_t = psingle.tile([128, 1], i32)
        nc.vector.memset(mask_t, 0x007FFFFF)
        rprime_t = psingle.tile([128, 1], f32)
        nc.vector.memset(rprime_t, RPRIME)

        for h in range(HPC):
            ksh, qsh, vsh = loads
            if h + 1 < HPC:
                # prefetch the next head's inputs behind this head's compute
                loads = emit_loads(h + 1)


            outc = pout.tile([65, S], f32)
            st = pst.tile([65, N_QC, 6], f32)

            e_tiles = {}  # (qc, g) -> e tile
            accs = {}     # qc -> (a1, a2)

            def emit_A(qc, g):
                """Scores + exp for group g of chunk qc."""
                gs, gl = GROUPS[g]
                L = gl * QC
                last = h == HPC - 1 and qc == N_QC - 1
                dve_groups = (
                    DVE_GROUPS_LAST
                    if last
                    else (DVE_GROUPS_EVEN if qc % 2 == 0 else DVE_GROUPS_ODD)
                )
                sc = psc.tile([128, 3 * QC], f32, tag=f"sc{g % 2}")
                for i in range(gs, gs + gl):
                    kb, half = divmod(i, 2)
                    col = (i - gs) * QC
                    ksk = ksh[kb // 8][:, ts(kb % 8, 128)]
                    nc.tensor.matmul(
                        sc[:, col : col + QC],
                        lhsT=ksk[64 * half : 64 * (half + 1), :],
                        rhs=qsh[qc][64 * half : 64 * (half + 1), :],
                        start=True,
                        stop=True,
                    )
                e = pe.tile([128, 3 * QC], f16, tag="e")
                e_tiles[(qc, g)] = e
                if g in dve_groups:
                    w32 = pw.tile([128, 3 * QC], i32, tag=f"w{g % 2}")
                    nc.vector.tensor_scalar(
                        out=w32[:, 0:L],
                        in0=sc[:, 0:L],
                        scalar1=A2,
                        scalar2=B2,
                        op0=OP.mult,
                        op1=OP.add,
                    )
                    nc.vector._custom_dve(
                        expop,
                        out=e[:, 0:L],
                        in0=w32[:, 0:L].bitcast(f32),
                        in1=rprime_t[:],
                        s0=mask_t[:].bitcast(f32),
                        s1=PPRIME,
                        imm2=QPRIME,
                    )
                else:
                    nc.scalar.activation(e[:, 0:L], sc[:, 0:L], AF.Exp, scale=SCALE)

            def emit_B(qc, g):
                """PV accumulation for group g of chunk qc."""
                gs, gl = GROUPS[g]
                a1, a2 = accs[qc]
                for i in range(gs, gs + gl):
                    kb, half = divmod(i, 2)
                    col = (i - gs) * QC
                    acc = a1 if half == 0 else a2
                    vk = vsh[half][kb // 8][:, ts(kb % 8, 65)]
                    nc.tensor.matmul(
                        acc[:],
                        lhsT=vk,
                        rhs=e_tiles.pop((qc, g))[:, col : col + QC]
                        if i == gs + gl - 1
                        else e_tiles[(qc, g)][:, col : col + QC],
                        start=(kb == 0),
                        stop=(kb == KB - 1),
                    )

            pending_evict = []
            pending_pre = []
            pending = []
            for qc in range(N_QC):
                last = h == HPC - 1 and qc == N_QC - 1
                if qc == 0:
                    emit_A(0, 0)
                    emit_A(0, 1)
                for g in range(2, len(GROUPS)):
                    if g == 2:
                        # previous chunk's accumulator evictions slot in here
                        # so they never head-of-line block this chunk's first
                        # activations on the ScalarE queue; the reciprocals
                        # follow and fill the DVE's early idle window
                        for fn in pending_evict:
                            fn()
                        pending_evict = []
                        for fn in pending_pre:
                            fn()
                        pending_pre = []
                    emit_A(qc, g)
                # previous chunk's deferred combine sits behind this chunk's
                # exp stream on the DVE
                for fn in pending:
                    fn()
                pending = []

                acc1 = pacc.tile([65, QC], f32, tag="a1")
                acc2 = pacc.tile([65, QC], f32, tag="a2")
                accs[qc] = (acc1, acc2)
                for g in range(0, 9):
                    emit_B(qc, g)
                # software-pipeline the chunk boundary: the next chunk's first
                # score groups go ahead of this chunk's tail PVs in the PE
                # stream so the activation rotation never stalls on them
                if qc + 1 < N_QC:
                    emit_A(qc + 1, 0)
                    emit_A(qc + 1, 1)
                for g in range(9, len(GROUPS)):
                    emit_B(qc, g)

                a1, a2 = accs.pop(qc)
                if last:
                    # banks are free after this — normalize straight out of
                    # PSUM, halves pipelined, to shorten the exposed tail
                    rb1r = pst.tile([1, QC], f32, tag="rb1r")
                    nc.vector.reciprocal_approx_fast(rb1r[:], a1[0:1, :])
                    rb1 = pep.tile([65, QC], f32, tag="rb1")
                    nc.gpsimd.partition_broadcast(rb1[:], rb1r[:], channels=65)
                    rb2r = pst.tile([1, QC], f32, tag="rb2r")
                    nc.vector.reciprocal_approx_fast(rb2r[:], a2[0:1, :])
                    rb2 = pep.tile([65, QC], f32, tag="rb2")
                    nc.gpsimd.partition_broadcast(rb2[:], rb2r[:], channels=65)
                    t1 = pep.tile([65, QC], f32, tag="lt1")
                    nc.vector.tensor_mul(t1[:], a1[:], rb1[:])
                    t2 = pep.tile([65, QC], f32, tag="lt2")
                    nc.vector.tensor_mul(t2[:], a2[:], rb2[:])
                    nc.vector.tensor_sub(outc[:, ts(qc, QC)], t1[:], t2[:])
                    nc.vector.bn_stats(st[:, qc, :], outc[:, ts(qc, QC)])
                    nc.sync.dma_start(outT[h, :, ts(qc, QC)], outc[1:65, ts(qc, QC)])
                else:
                    # both halves side by side in one staging tile so the
                    # deferred normalize is one wide op per stage
                    sa = psa.tile([65, 2 * QC], f32, tag=f"sa_{qc % 2}")

                    def _evict(a1=a1, a2=a2, sa=sa):
                        nc.scalar.copy(sa[:, 0:QC], a1[:])
                        nc.scalar.copy(sa[:, QC:], a2[:])

                    def _recips(sa=sa):
                        rbr = pst.tile([1, 2 * QC], f32, tag="rbr")
                        nc.vector.reciprocal_approx_fast(rbr[:], sa[0:1, :])
                        rb = pep.tile([65, 2 * QC], f32, tag="rb")
                        nc.gpsimd.partition_broadcast(rb[:], rbr[:], channels=65)
                        return rb

                    def _combine(rb, qc=qc, sa=sa, h=h):
                        tt = pep.tile([65, 2 * QC], f32, tag=f"tt_{qc % 2}")
                        nc.vector.tensor_mul(tt[:], sa[:], rb[:])
                        nc.vector.tensor_sub(
                            outc[:, ts(qc, QC)], tt[:, 0:QC], tt[:, QC:]
                        )
                        nc.vector.bn_stats(st[:, qc, :], outc[:, ts(qc, QC)])
                        # un-affined diff streams out now; the host applies
                        # the per-head GroupNorm affine during unsharding
                        nc.sync.dma_start(
                            outT[h, :, ts(qc, QC)], outc[1:65, ts(qc, QC)]
                        )

                    pending_evict.append(_evict)
                    rbs = []
                    pending_pre.append(lambda r=rbs, f=_recips: r.append(f()))
                    pending.append(lambda r=rbs, f=_combine: f(r[0]))
            for fn in pending_evict:
                fn()
            pending_evict = []
            for fn in pending_pre:
                fn()
            pending_pre = []
            for fn in pending:
                fn()
            pending = []

            # ---- head finalize: per-partition (mean, var) over S leave the
            # device; the 64-way scalar reduction + rsqrt + affine happen on
            # the host during unsharding ----
            mv = pst.tile([65, 2], f32)
            nc.vector.bn_aggr(mv[:], st[:])
            nc.sync.dma_start(sgtb[h], mv[:])

    nc.compile()
    return nc


def _get_nc():
    if "nc" not in _CACHE:
        _CACHE["nc"] = _build_nc()
    return _CACHE["nc"]


def _host_prep(q, k, v, lq1, lq2, lk1, lk2, gamma, beta):
    """Build per-core input maps."""
    q = np.asarray(q, dtype=np.float32)
    k = np.asarray(k, dtype=np.float32)
    v = np.asarray(v, dtype=np.float32)
    lam = float(
        np.exp(np.float32(np.dot(lq1, lk1)))
        - np.exp(np.float32(np.dot(lq2, lk2)))
        + LAMBDA_INIT
    )
    g2 = (np.asarray(gamma, np.float32) * (1.0 - LAMBDA_INIT)).reshape(H, D)
    b2 = (np.asarray(beta, np.float32) * (1.0 - LAMBDA_INIT)).reshape(H, D)

    in_maps = []
    for c in range(N_CORES):
        heads = range(c * HPC, (c + 1) * HPC)
        qTa = np.empty((HPC, 128, S), np.float16)
        kTa = np.empty((HPC, 128, S), np.float16)
        vpa = np.empty((HPC, 2, 128, KB * 65), np.float16)
        gba = np.empty((HPC, 64, 2), np.float32)
        for i, hh in enumerate(heads):
            qTa[i] = q[0, hh].T.astype(np.float16)
            kTa[i] = k[0, hh].T.astype(np.float16)
            vh = v[0, hh]  # [S, 64]
            v1 = np.concatenate([np.ones((S, 1), np.float32), vh], axis=1)
            v2 = np.concatenate([np.ones((S, 1), np.float32), lam * vh], axis=1)
            # SBUF image: [partition(key within block), kblock*65 + col]
            vpa[i, 0] = (
                v1.reshape(KB, 128, 65).transpose(1, 0, 2).reshape(128, KB * 65)
            ).astype(np.float16)
            vpa[i, 1] = (
                v2.reshape(KB, 128, 65).transpose(1, 0, 2).reshape(128, KB * 65)
            ).astype(np.float16)
            gba[i, :, 0] = g2[hh]
            gba[i, :, 1] = b2[hh]
        in_maps.append({"qT": qTa, "kT": kTa, "vp": vpa, "gb": gba})
    return in_maps


def kernel(q, k, v, lq1, lq2, lk1, lk2, gamma, beta, _trace=False, _tmpdir=None):
    from concourse.bass_utils import run_bass_kernel_spmd

    nc = _get_nc()
    in_maps = _host_prep(q, k, v, lq1, lq2, lk1, lk2, gamma, beta)
    res = run_bass_kernel_spmd(
        nc,
        in_maps,
        core_ids=list(range(N_CORES)),
        trace=_trace,
        tmpdir=_tmpdir,
    )
    g2 = (np.asarray(gamma, np.float32) * (1.0 - LAMBDA_INIT)).reshape(H, D)
    b2 = (np.asarray(beta, np.float32) * (1.0 - LAMBDA_INIT)).reshape(H, D)
    out = np.empty((B, H, S, D), np.float32)
    for c in range(N_CORES):
        outT = res.results[c]["outT"]  # [HPC, 64, S] un-affined diff
        mvs = res.results[c]["sgtb"]   # [HPC, 65, 2] per-partition (mean, var)
        for i in range(HPC):
            hh = c * HPC + i
            mean_p = mvs[i, 1:65, 0].astype(np.float64)
            var_p = mvs[i, 1:65, 1].astype(np.float64)
            mu = mean_p.mean()
            var = (var_p + mean_p**2).mean() - mu * mu
            rstd = 1.0 / np.sqrt(var + EPS)
            sg = (rstd * g2[hh]).astype(np.float32)[:, None]
            tb = (b2[hh] - mu * rstd * g2[hh]).astype(np.float32)[:, None]
            out[0, hh] = (outT[i] * sg + tb).T
    if _trace:
        _CACHE["last_results"] = res
    return out


# revision 54
# speedup vs baseline: 1.0150x; 1.0066x over previous
"""Differential attention (two-softmax diff + GroupNorm) on 8 TRN2 cores.

Sharding: 16 heads / 8 cores = 2 heads per core (head-parallel, no
collectives). GroupNorm stats are per-(batch, head) so each core is fully
independent.

Device layout (host prepares everything):
  - Q, K per head are host-transposed to [128(d), 2048(s)] fp16: partitions
    0-63 hold half-1 (q1/k1), partitions 64-127 hold half-2. QK^T contracts
    over the partition dim, producing transposed score blocks S^T[key, query]
    in PSUM (fp32). The two 64-contraction halves auto-derive PE row-tile
    positions (0,0)/(64,0) and execute concurrently on the array.
  - V per head is prefixed with a ones column (V' = [1 | V], 65 cols, fp16)
    pre-arranged as [128(key-in-block), 16*65]: the PV matmul yields the
    softmax denominator on partition 0 and the numerator on partitions 1-64.
    lam is folded into half-2's V on the host.

ScalarE's exp over 2*S^2 scores/head is the bottleneck engine (~1.1 ns per
lane-element + ~260ns fixed per ACTIVATE). Levers:
  1. Bigger activation batches: scores accumulate in two rotating 3-bank
     PSUM tiles [128, 1536] so each ACTIVATE covers 3 slabs.
  2. ~25% of slab-groups are exp'd on the VectorE via a 2-instruction
     Schraudolph pipeline with cubic mantissa correction (max rel err
     ~7.7e-4, same class as the fp16 store quantization):
       i   = round_f32(s*A2 + B2)      stock tensor_scalar, f32->i32 convert
       w   = bitcast_f32(i)            = 2^(t+delta) * (1+f)/2^f
       m   = (i & 0x7FFFFF) | One.bits custom op: mantissa -> [1,2)
       e   = (((p'-m)m + q')m + r') * w   cubic corr, |c3| folded into B2
     The finisher is one 8-stage custom-DVE op (and, or, sub, mul, add,
     mul, add, mul) registered at import time.

Scheduling: per chunk, pass A emits all QK + exp (e production runs ahead),
pass B the serial PV accumulation; the next chunk's first two score groups
are emitted before this chunk's tail PVs (software-pipelined boundary).
Accumulators are evicted by ScalarE copies slotted into the next chunk's
act stream; the normalize (reciprocal_approx_fast on the den rows, GpSimd
partition broadcasts, multiply/subtract) is deferred behind the next
chunk's exp work on the DVE. bn stats stay on-device per chunk; the final
64-way scalar stat reduction, rsqrt and GroupNorm affine are applied on
the host during unsharding (outT carries the un-affined diff, sgtb the
per-partition (mean, var)).
"""

import math

import numpy as np

B, H, S, D = 1, 16, 2048, 64
N_CORES = 8
HPC = H // N_CORES  # heads per core
QC = 512            # query-chunk width
N_QC = S // QC
KB = S // 128       # key blocks of 128
LAMBDA_INIT = 0.8
EPS = 1e-5
SCALE = 1.0 / math.sqrt(D)
N_WARMUP_MM = 14

# cubic minimax fit of R(m) = 2^(m-1)/m on [1,2):  c3 m^3 + c2 m^2 + c1 m + c0
_C3 = -0.10246085749846692
_C2 = 0.69063801
_C1 = -1.35417106
_C0 = 1.76527539
PPRIME = -_C2 / _C3            # +6.7405058
QPRIME = -_C1 / _C3            # -13.216472
RPRIME = -_C0 / _C3            # +17.228778
DELTA = math.log2(-_C3)        # fold |c3| into the exponent bias
A2 = float(np.float32(math.log2(math.e) * SCALE * 2.0**23))
B2 = float(np.float32((127.0 + DELTA) * 2.0**23))

# slab-groups per 512-query chunk: 32 slabs of [128,512] scores -> 11 groups
GROUPS = [(i * 3, 3) for i in range(10)] + [(30, 2)]
# group indices handled by the VectorE exp pipeline (rest: ScalarE ACTIVATE);
# mid placement keeps the e-latency off both the rotation head and the PV
# chain tail; alternation balances the two engines at ~2.5 groups/chunk
DVE_GROUPS_EVEN = (2, 5, 8)
DVE_GROUPS_ODD = (3, 7)
DVE_GROUPS_LAST = (3, 7)

_CACHE = {}


def _get_exp_op():
    """Register (once) and return the custom-DVE exp-finisher op."""
    if "expop" in _CACHE:
        return _CACHE["expop"]
    from concourse import dve_ops
    from concourse.dve_spec import (
        AluOp,
        Bin,
        C0,
        C1,
        C2,
        C3,
        One,
        Spec,
        Src0,
        _spill_c3_to_src1,
        lower,
    )
    from concourse.dve_uop import DveOpSpec

    for existing in dve_ops.OPS:
        if existing.name == "ANT_EXP2_FINISH":
            _CACHE["expop"] = existing
            return existing

    mm = Bin(AluOp.BITWISE_AND, Src0, C0)
    mo = Bin(AluOp.BITWISE_OR, mm, One)
    t5 = ((C1 - mo) * mo + C2) * mo + C3
    body = _spill_c3_to_src1(t5 * Src0)

    def _ref(in0, in1, s0, s1, imm2):
        bits = np.asarray(in0, np.float32).view(np.int32)
        s0i = np.asarray(s0).view(np.int32) if isinstance(s0, np.ndarray) else np.int32(s0)
        m = ((bits & s0i) | np.int32(0x3F800000)).view(np.float32)
        t = ((np.float32(s1) - m) * m + np.float32(imm2)) * m + np.asarray(
            in1, np.float32
        )
        return t * np.asarray(in0, np.float32)

    spec = Spec(body=body, reference=_ref)
    op = dve_ops.DveOp("ANT_EXP2_FINISH", spec, subdim=False, uops_sha={})
    dve_ops.OPS.append(op)
    dve_ops._SUB_OPCODE_FOR_NAME[op.name] = dve_ops._CUSTOM_DVE_ROW_BASE + len(
        dve_ops.OPS
    ) - 1
    dve_ops.CUSTOM_DVE_SPECS[op.name] = spec
    for ver in ("v3", "v4"):
        tmp = DveOpSpec(
            name=op.name,
            opcode=dve_ops.get_dve_sub_opcode(op.name),
            uops=lower(spec, ver=ver),
            rd1_en=True,
        )
        op.uops_sha[ver] = tmp.sha(ver)
    _CACHE["expop"] = op
    return op


def _build_nc():
    from contextlib import ExitStack

    import concourse.bacc as bacc
    import concourse.bass as bass
    import concourse.tile as tile
    from concourse import bass_isa, mybir

    f32 = mybir.dt.float32
    f16 = mybir.dt.float16
    i32 = mybir.dt.int32
    AF = mybir.ActivationFunctionType
    OP = mybir.AluOpType
    ts = bass.ts

    expop = _get_exp_op()

    nc = bacc.Bacc("TRN2", target_bir_lowering=False, debug=False)

    qT = nc.dram_tensor("qT", [HPC, 128, S], f16, kind="ExternalInput").ap()
    kT = nc.dram_tensor("kT", [HPC, 128, S], f16, kind="ExternalInput").ap()
    vp = nc.dram_tensor("vp", [HPC, 2, 128, KB * 65], f16, kind="ExternalInput").ap()
    gb = nc.dram_tensor("gb", [HPC, 64, 2], f32, kind="ExternalInput").ap()
    outT = nc.dram_tensor("outT", [HPC, 64, S], f32, kind="ExternalOutput").ap()
    sgtb = nc.dram_tensor("sgtb", [HPC, 65, 2], f32, kind="ExternalOutput").ap()

    with tile.TileContext(nc) as tc, ExitStack() as ctx:
        pq = ctx.enter_context(tc.tile_pool(name="pq", bufs=2))
        pk = ctx.enter_context(tc.tile_pool(name="pk", bufs=2))
        pv = ctx.enter_context(tc.tile_pool(name="pv", bufs=2))
        pe = ctx.enter_context(tc.tile_pool(name="pe", bufs=13))
        pw = ctx.enter_context(tc.tile_pool(name="pw", bufs=1))
        pep = ctx.enter_context(tc.tile_pool(name="pep", bufs=2))
        psa = ctx.enter_context(tc.tile_pool(name="psa", bufs=2))
        pout = ctx.enter_context(tc.tile_pool(name="pout", bufs=2))
        pst = ctx.enter_context(tc.tile_pool(name="pst", bufs=2))
        psingle = ctx.enter_context(tc.tile_pool(name="psingle", bufs=1))
        psc = ctx.enter_context(tc.tile_pool(name="psc", bufs=1, space="PSUM"))
        pacc = ctx.enter_context(tc.tile_pool(name="pacc", bufs=1, space="PSUM"))

        def emit_loads(h):
            """DMA the head's inputs; split so the first matmuls start early."""
            ksh = []
            for j in range(2):
                ks_t = pk.tile([128, S // 2], f16, tag=f"ks{j}", name=f"ks{j}")
                ksh.append(ks_t)
            qsh = []
            for j in range(N_QC):
                qs_t = pq.tile([128, QC], f16, tag=f"qs{j}", name=f"qs{j}")
                qsh.append(qs_t)
            nc.sync.dma_start(ksh[0][:], kT[h, :, 0 : S // 2])
            nc.sync.dma_start(qsh[0][:], qT[h, :, 0:QC])
            nc.sync.dma_start(ksh[1][:], kT[h, :, S // 2 : S])
            for j in range(1, N_QC):
                nc.sync.dma_start(qsh[j][:], qT[h, :, j * QC : (j + 1) * QC])
            vsh = []
            for half in range(2):
                row = []
                for j in range(2):
                    t = pv.tile(
                        [128, KB * 65 // 2], f16, tag=f"v{half}{j}", name=f"v{half}{j}"
                    )
                    nc.sync.dma_start(
                        t[:],
                        vp[h, half, :, j * (KB * 65 // 2) : (j + 1) * (KB * 65 // 2)],
                    )
                    row.append(t)
                vsh.append(row)
            return ksh, qsh, vsh

        # PE warm-up: tiny back-to-back matmuls flip the HAM clock gate to
        # 8/8 while the first head's DMAs are in flight.
        wu_w = psingle.tile([128, 128], f16)
        nc.vector.memset(wu_w, 0.0)
        wu_ps = psc.tile([128, 3 * QC], f32, tag="sc0")
        for _ in range(N_WARMUP_MM):
            nc.tensor.matmul(
                wu_ps[:, 0:128], lhsT=wu_w[:], rhs=wu_w[:], start=True, stop=True
            )

        loads = emit_loads(0)

        mask_t = psingle.tile([128, 1], i32)
        nc.vector.memset(mask_t, 0x007FFFFF)
        rprime_t = psingle.tile([128, 1], f32)
        nc.vector.memset(rprime_t, RPRIME)

        for h in range(HPC):
            ksh, qsh, vsh = loads
            if h + 1 < HPC:
                # prefetch the next head's inputs behind this head's compute
                loads = emit_loads(h + 1)


            outc = pout.tile([65, S], f32)
            st = pst.tile([65, N_QC, 6], f32)

            e_tiles = {}  # (qc, g) -> e tile
            accs = {}     # qc -> (a1, a2)

            def emit_A(qc, g):
                """Scores + exp for group g of chunk qc."""
                gs, gl = GROUPS[g]
                L = gl * QC
                last = h == HPC - 1 and qc == N_QC - 1
                dve_groups = (
                    DVE_GROUPS_LAST
                    if last
                    else (DVE_GROUPS_EVEN if qc % 2 == 0 else DVE_GROUPS_ODD)
                )
                sc = psc.tile([128, 3 * QC], f32, tag=f"sc{g % 2}")
                for i in range(gs, gs + gl):
                    kb, half = divmod(i, 2)
                    col = (i - gs) * QC
                    ksk = ksh[kb // 8][:, ts(kb % 8, 128)]
                    nc.tensor.matmul(
                        sc[:, col : col + QC],
                        lhsT=ksk[64 * half : 64 * (half + 1), :],
                        rhs=qsh[qc][64 * half : 64 * (half + 1), :],
                        start=True,
                        stop=True,
                    )
                e = pe.tile([128, 3 * QC], f16, tag="e")
                e_tiles[(qc, g)] = e
                if g in dve_groups:
                    w32 = pw.tile([128, 3 * QC], i32, tag=f"w{g % 2}")
                    nc.vector.tensor_scalar(
                        out=w32[:, 0:L],
                        in0=sc[:, 0:L],
                        scalar1=A2,
                        scalar2=B2,
                        op0=OP.mult,
                        op1=OP.add,
                    )
                    nc.vector._custom_dve(
                        expop,
                        out=e[:, 0:L],
                        in0=w32[:, 0:L].bitcast(f32),
                        in1=rprime_t[:],
                        s0=mask_t[:].bitcast(f32),
                        s1=PPRIME,
                        imm2=QPRIME,
                    )
                else:
                    nc.scalar.activation(e[:, 0:L], sc[:, 0:L], AF.Exp, scale=SCALE)

            def emit_B(qc, g):
                """PV accumulation for group g of chunk qc."""
                gs, gl = GROUPS[g]
                a1, a2 = accs[qc]
                for i in range(gs, gs + gl):
                    kb, half = divmod(i, 2)
                    col = (i - gs) * QC
                    acc = a1 if half == 0 else a2
                    vk = vsh[half][kb // 8][:, ts(kb % 8, 65)]
                    nc.tensor.matmul(
                        acc[:],
                        lhsT=vk,
                        rhs=e_tiles.pop((qc, g))[:, col : col + QC]
                        if i == gs + gl - 1
                        else e_tiles[(qc, g)][:, col : col + QC],
                        start=(kb == 0),
                        stop=(kb == KB - 1),
                    )

            pending_evict = []
            pending_pre = []
            pending = []
            pending_next = []
            for qc in range(N_QC):
                last = h == HPC - 1 and qc == N_QC - 1
                if qc == 0:
                    emit_A(0, 0)
                    emit_A(0, 1)
                for g in range(2, len(GROUPS)):
                    if g == 2:
                        # previous chunk's accumulator evictions slot in here
                        # so they never head-of-line block this chunk's first
                        # activations on the ScalarE queue
                        for fn in pending_evict:
                            fn()
                        pending_evict = []
                    emit_A(qc, g)
                # deferred normalize, double-staggered so neither the copy
                # wait (recips) nor the broadcast wait (combine) ever sits at
                # the DVE FIFO head in front of exp work: reciprocals of
                # chunk qc-1 and combines of chunk qc-2 run here
                for fn in pending_pre:
                    fn()
                pending_pre = []
                for fn in pending:
                    fn()
                pending = pending_next
                pending_next = []

                acc1 = pacc.tile([65, QC], f32, tag="a1")
                acc2 = pacc.tile([65, QC], f32, tag="a2")
                accs[qc] = (acc1, acc2)
                for g in range(0, 9):
                    emit_B(qc, g)
                # software-pipeline the chunk boundary: the next chunk's first
                # score groups go ahead of this chunk's tail PVs in the PE
                # stream so the activation rotation never stalls on them
                if qc + 1 < N_QC:
                    emit_A(qc + 1, 0)
                    emit_A(qc + 1, 1)
                for g in range(9, len(GROUPS)):
                    emit_B(qc, g)

                a1, a2 = accs.pop(qc)
                if last:
                    # banks are free after this — normalize straight out of
                    # PSUM, halves pipelined, to shorten the exposed tail
                    rb1r = pst.tile([1, QC], f32, tag="rb1r")
                    nc.vector.reciprocal_approx_fast(rb1r[:], a1[0:1, :])
                    rb1 = pep.tile([65, QC], f32, tag="rb1")
                    nc.gpsimd.partition_broadcast(rb1[:], rb1r[:], channels=65)
                    rb2r = pst.tile([1, QC], f32, tag="rb2r")
                    nc.vector.reciprocal_approx_fast(rb2r[:], a2[0:1, :])
                    rb2 = pep.tile([65, QC], f32, tag="rb2")
                    nc.gpsimd.partition_broadcast(rb2[:], rb2r[:], channels=65)
                    t1 = pep.tile([65, QC], f32, tag="lt1")
                    nc.vector.tensor_mul(t1[:], a1[:], rb1[:])
                    t2 = pep.tile([65, QC], f32, tag="lt2")
                    nc.vector.tensor_mul(t2[:], a2[:], rb2[:])
                    nc.vector.tensor_sub(outc[:, ts(qc, QC)], t1[:], t2[:])
                    nc.vector.bn_stats(st[:, qc, :], outc[:, ts(qc, QC)])
                    nc.sync.dma_start(outT[h, :, ts(qc, QC)], outc[1:65, ts(qc, QC)])
                else:
                    # both halves side by side in one staging tile so the
                    # deferred normalize is one wide op per stage
                    sa = psa.tile([65, 2 * QC], f32, tag=f"sa_{qc % 2}")

                    def _evict(a1=a1, a2=a2, sa=sa):
                        nc.scalar.copy(sa[:, 0:QC], a1[:])
                        nc.scalar.copy(sa[:, QC:], a2[:])

                    def _recips(sa=sa):
                        rbr = pst.tile([1, 2 * QC], f32, tag="rbr")
                        nc.vector.reciprocal_approx_fast(rbr[:], sa[0:1, :])
                        rb = pep.tile([65, 2 * QC], f32, tag="rb")
                        nc.gpsimd.partition_broadcast(rb[:], rbr[:], channels=65)
                        return rb

                    def _combine(rb, qc=qc, sa=sa, h=h):
                        tt = pep.tile([65, 2 * QC], f32, tag=f"tt_{qc % 2}")
                        nc.vector.tensor_mul(tt[:], sa[:], rb[:])
                        nc.vector.tensor_sub(
                            outc[:, ts(qc, QC)], tt[:, 0:QC], tt[:, QC:]
                        )
                        nc.vector.bn_stats(st[:, qc, :], outc[:, ts(qc, QC)])
                        # un-affined diff streams out now; the host applies
                        # the per-head GroupNorm affine during unsharding
                        nc.sync.dma_start(
                            outT[h, :, ts(qc, QC)], outc[1:65, ts(qc, QC)]
                        )

                    pending_evict.append(_evict)
                    rbs = []
                    pending_pre.append(lambda r=rbs, f=_recips: r.append(f()))
                    pending_next.append(lambda r=rbs, f=_combine: f(r[0]))
            for fn in pending_evict:
                fn()
            pending_evict = []
            for fn in pending_pre:
                fn()
            pending_pre = []
            for fn in pending:
                fn()
            pending = []
            for fn in pending_next:
                fn()
            pending_next = []

            # ---- head finalize: per-partition (mean, var) over S leave the
            # device; the 64-way scalar reduction + rsqrt + affine happen on
            # the host during unsharding ----
            mv = pst.tile([65, 2], f32)
            nc.vector.bn_aggr(mv[:], st[:])
            nc.sync.dma_start(sgtb[h], mv[:])

    nc.compile()
    return nc


def _get_nc():
    if "nc" not in _CACHE:
        _CACHE["nc"] = _build_nc()
    return _CACHE["nc"]


def _host_prep(q, k, v, lq1, lq2, lk1, lk2, gamma, beta):
    """Build per-core input maps."""
    q = np.asarray(q, dtype=np.float32)
    k = np.asarray(k, dtype=np.float32)
    v = np.asarray(v, dtype=np.float32)
    lam = float(
        np.exp(np.float32(np.dot(lq1, lk1)))
        - np.exp(np.float32(np.dot(lq2, lk2)))
        + LAMBDA_INIT
    )
    g2 = (np.asarray(gamma, np.float32) * (1.0 - LAMBDA_INIT)).reshape(H, D)
    b2 = (np.asarray(beta, np.float32) * (1.0 - LAMBDA_INIT)).reshape(H, D)

    in_maps = []
    for c in range(N_CORES):
        heads = range(c * HPC, (c + 1) * HPC)
        qTa = np.empty((HPC, 128, S), np.float16)
        kTa = np.empty((HPC, 128, S), np.float16)
        vpa = np.empty((HPC, 2, 128, KB * 65), np.float16)
        gba = np.empty((HPC, 64, 2), np.float32)
        for i, hh in enumerate(heads):
            qTa[i] = q[0, hh].T.astype(np.float16)
            kTa[i] = k[0, hh].T.astype(np.float16)
            vh = v[0, hh]  # [S, 64]
            v1 = np.concatenate([np.ones((S, 1), np.float32), vh], axis=1)
            v2 = np.concatenate([np.ones((S, 1), np.float32), lam * vh], axis=1)
            # SBUF image: [partition(key within block), kblock*65 + col]
            vpa[i, 0] = (
                v1.reshape(KB, 128, 65).transpose(1, 0, 2).reshape(128, KB * 65)
            ).astype(np.float16)
            vpa[i, 1] = (
                v2.reshape(KB, 128, 65).transpose(1, 0, 2).reshape(128, KB * 65)
            ).astype(np.float16)
            gba[i, :, 0] = g2[hh]
            gba[i, :, 1] = b2[hh]
        in_maps.append({"qT": qTa, "kT": kTa, "vp": vpa, "gb": gba})
    return in_maps


def kernel(q, k, v, lq1, lq2, lk1, lk2, gamma, beta, _trace=False, _tmpdir=None):
    from concourse.bass_utils import run_bass_kernel_spmd

    nc = _get_nc()
    in_maps = _host_prep(q, k, v, lq1, lq2, lk1, lk2, gamma, beta)
    res = run_bass_kernel_spmd(
        nc,
        in_maps,
        core_ids=list(range(N_CORES)),
        trace=_trace,
        tmpdir=_tmpdir,
    )
    g2 = (np.asarray(gamma, np.float32) * (1.0 - LAMBDA_INIT)).reshape(H, D)
    b2 = (np.asarray(beta, np.float32) * (1.0 - LAMBDA_INIT)).reshape(H, D)
    out = np.empty((B, H, S, D), np.float32)
    for c in range(N_CORES):
        outT = res.results[c]["outT"]  # [HPC, 64, S] un-affined diff
        mvs = res.results[c]["sgtb"]   # [HPC, 65, 2] per-partition (mean, var)
        for i in range(HPC):
            hh = c * HPC + i
            mean_p = mvs[i, 1:65, 0].astype(np.float64)
            var_p = mvs[i, 1:65, 1].astype(np.float64)
            mu = mean_p.mean()
            var = (var_p + mean_p**2).mean() - mu * mu
            rstd = 1.0 / np.sqrt(var + EPS)
            sg = (rstd * g2[hh]).astype(np.float32)[:, None]
            tb = (b2[hh] - mu * rstd * g2[hh]).astype(np.float32)[:, None]
            out[0, hh] = (outT[i] * sg + tb).T
    if _trace:
        _CACHE["last_results"] = res
    return out


# revision 56
# speedup vs baseline: 1.0339x; 1.0187x over previous
"""Differential attention (two-softmax diff + GroupNorm) on 8 TRN2 cores.

Sharding: 16 heads / 8 cores = 2 heads per core (head-parallel, no
collectives). GroupNorm stats are per-(batch, head) so each core is fully
independent.

Device layout (host prepares everything):
  - Q, K per head are host-transposed to [128(d), 2048(s)] fp16: partitions
    0-63 hold half-1 (q1/k1), partitions 64-127 hold half-2. QK^T contracts
    over the partition dim, producing transposed score blocks S^T[key, query]
    in PSUM (fp32). The two 64-contraction halves auto-derive PE row-tile
    positions (0,0)/(64,0) and execute concurrently on the array.
  - V per head is prefixed with a ones column (V' = [1 | V], 65 cols, fp16)
    pre-arranged as [128(key-in-block), 16*65]: the PV matmul yields the
    softmax denominator on partition 0 and the numerator on partitions 1-64.
    lam is folded into half-2's V on the host.

ScalarE's exp over 2*S^2 scores/head is the bottleneck engine (~1.1 ns per
lane-element + ~260ns fixed per ACTIVATE). Levers:
  1. Bigger activation batches: scores accumulate in two rotating 3-bank
     PSUM tiles [128, 1536] so each ACTIVATE covers 3 slabs.
  2. ~25% of slab-groups are exp'd on the VectorE via a 2-instruction
     Schraudolph pipeline with cubic mantissa correction (max rel err
     ~7.7e-4, same class as the fp16 store quantization):
       i   = round_f32(s*A2 + B2)      stock tensor_scalar, f32->i32 convert
       w   = bitcast_f32(i)            = 2^(t+delta) * (1+f)/2^f
       m   = (i & 0x7FFFFF) | One.bits custom op: mantissa -> [1,2)
       e   = (((p'-m)m + q')m + r') * w   cubic corr, |c3| folded into B2
     The finisher is one 8-stage custom-DVE op (and, or, sub, mul, add,
     mul, add, mul) registered at import time.

Scheduling: per chunk, pass A emits all QK + exp (e production runs ahead),
pass B the serial PV accumulation; the next chunk's first two score groups
are emitted before this chunk's tail PVs (software-pipelined boundary).
Accumulators are evicted by ScalarE copies slotted into the next chunk's
act stream; the normalize (reciprocal_approx_fast on the den rows, GpSimd
partition broadcasts, multiply/subtract) is deferred behind the next
chunk's exp work on the DVE. bn stats stay on-device per chunk; the final
64-way scalar stat reduction, rsqrt and GroupNorm affine are applied on
the host during unsharding (outT carries the un-affined diff, sgtb the
per-partition (mean, var)).
"""

import math

import numpy as np

B, H, S, D = 1, 16, 2048, 64
N_CORES = 8
HPC = H // N_CORES  # heads per core
QC = 512            # query-chunk width
N_QC = S // QC
KB = S // 128       # key blocks of 128
LAMBDA_INIT = 0.8
EPS = 1e-5
SCALE = 1.0 / math.sqrt(D)
N_WARMUP_MM = 14

# cubic minimax fit of R(m) = 2^(m-1)/m on [1,2):  c3 m^3 + c2 m^2 + c1 m + c0
_C3 = -0.10246085749846692
_C2 = 0.69063801
_C1 = -1.35417106
_C0 = 1.76527539
PPRIME = -_C2 / _C3            # +6.7405058
QPRIME = -_C1 / _C3            # -13.216472
RPRIME = -_C0 / _C3            # +17.228778
DELTA = math.log2(-_C3)        # fold |c3| into the exponent bias
A2 = float(np.float32(math.log2(math.e) * SCALE * 2.0**23))
B2 = float(np.float32((127.0 + DELTA) * 2.0**23))

# slab-groups per 512-query chunk: 32 slabs of [128,512] scores -> 11 groups
GROUPS = [(i * 3, 3) for i in range(10)] + [(30, 2)]
# group indices handled by the VectorE exp pipeline (rest: ScalarE ACTIVATE);
# mid placement keeps the e-latency off both the rotation head and the PV
# chain tail; alternation balances the two engines at ~2.5 groups/chunk
DVE_GROUPS_EVEN = (2, 5, 8)
DVE_GROUPS_ODD = (3, 7)
DVE_GROUPS_LAST = (3, 7)

_CACHE = {}


def _get_exp_op():
    """Register (once) and return the custom-DVE exp-finisher op."""
    if "expop" in _CACHE:
        return _CACHE["expop"]
    from concourse import dve_ops
    from concourse.dve_spec import (
        AluOp,
        Bin,
        C0,
        C1,
        C2,
        C3,
        One,
        Spec,
        Src0,
        _spill_c3_to_src1,
        lower,
    )
    from concourse.dve_uop import DveOpSpec

    for existing in dve_ops.OPS:
        if existing.name == "ANT_EXP2_FINISH":
            _CACHE["expop"] = existing
            return existing

    mm = Bin(AluOp.BITWISE_AND, Src0, C0)
    mo = Bin(AluOp.BITWISE_OR, mm, One)
    t5 = ((C1 - mo) * mo + C2) * mo + C3
    body = _spill_c3_to_src1(t5 * Src0)

    def _ref(in0, in1, s0, s1, imm2):
        bits = np.asarray(in0, np.float32).view(np.int32)
        s0i = np.asarray(s0).view(np.int32) if isinstance(s0, np.ndarray) else np.int32(s0)
        m = ((bits & s0i) | np.int32(0x3F800000)).view(np.float32)
        t = ((np.float32(s1) - m) * m + np.float32(imm2)) * m + np.asarray(
            in1, np.float32
        )
        return t * np.asarray(in0, np.float32)

    spec = Spec(body=body, reference=_ref)
    op = dve_ops.DveOp("ANT_EXP2_FINISH", spec, subdim=False, uops_sha={})
    dve_ops.OPS.append(op)
    dve_ops._SUB_OPCODE_FOR_NAME[op.name] = dve_ops._CUSTOM_DVE_ROW_BASE + len(
        dve_ops.OPS
    ) - 1
    dve_ops.CUSTOM_DVE_SPECS[op.name] = spec
    for ver in ("v3", "v4"):
        tmp = DveOpSpec(
            name=op.name,
            opcode=dve_ops.get_dve_sub_opcode(op.name),
            uops=lower(spec, ver=ver),
            rd1_en=True,
        )
        op.uops_sha[ver] = tmp.sha(ver)
    _CACHE["expop"] = op
    return op


def _build_nc():
    from contextlib import ExitStack

    import concourse.bacc as bacc
    import concourse.bass as bass
    import concourse.tile as tile
    from concourse import bass_isa, mybir

    f32 = mybir.dt.float32
    f16 = mybir.dt.float16
    i32 = mybir.dt.int32
    AF = mybir.ActivationFunctionType
    OP = mybir.AluOpType
    ts = bass.ts

    expop = _get_exp_op()

    nc = bacc.Bacc("TRN2", target_bir_lowering=False, debug=False)

    qT = nc.dram_tensor("qT", [HPC, 128, S], f16, kind="ExternalInput").ap()
    kT = nc.dram_tensor("kT", [HPC, 128, S], f16, kind="ExternalInput").ap()
    vp = nc.dram_tensor("vp", [HPC, 2, 128, KB * 65], f16, kind="ExternalInput").ap()
    gb = nc.dram_tensor("gb", [HPC, 64, 2], f32, kind="ExternalInput").ap()
    outT = nc.dram_tensor("outT", [HPC, 64, S], f32, kind="ExternalOutput").ap()
    sgtb = nc.dram_tensor("sgtb", [HPC, 65, 2], f32, kind="ExternalOutput").ap()

    with tile.TileContext(nc) as tc, ExitStack() as ctx:
        pq = ctx.enter_context(tc.tile_pool(name="pq", bufs=2))
        pk = ctx.enter_context(tc.tile_pool(name="pk", bufs=2))
        pv = ctx.enter_context(tc.tile_pool(name="pv", bufs=2))
        pe = ctx.enter_context(tc.tile_pool(name="pe", bufs=13))
        pw = ctx.enter_context(tc.tile_pool(name="pw", bufs=1))
        pep = ctx.enter_context(tc.tile_pool(name="pep", bufs=2))
        psa = ctx.enter_context(tc.tile_pool(name="psa", bufs=2))
        pout = ctx.enter_context(tc.tile_pool(name="pout", bufs=2))
        pst = ctx.enter_context(tc.tile_pool(name="pst", bufs=2))
        psingle = ctx.enter_context(tc.tile_pool(name="psingle", bufs=1))
        psc = ctx.enter_context(tc.tile_pool(name="psc", bufs=1, space="PSUM"))
        pacc = ctx.enter_context(tc.tile_pool(name="pacc", bufs=1, space="PSUM"))

        def emit_loads(h):
            """DMA the head's inputs; split so the first matmuls start early."""
            ksh = []
            for j in range(2):
                ks_t = pk.tile([128, S // 2], f16, tag=f"ks{j}", name=f"ks{j}")
                ksh.append(ks_t)
            qsh = []
            for j in range(N_QC):
                qs_t = pq.tile([128, QC], f16, tag=f"qs{j}", name=f"qs{j}")
                qsh.append(qs_t)
            nc.sync.dma_start(ksh[0][:], kT[h, :, 0 : S // 2])
            nc.sync.dma_start(qsh[0][:], qT[h, :, 0:QC])
            nc.sync.dma_start(ksh[1][:], kT[h, :, S // 2 : S])
            for j in range(1, N_QC):
                nc.sync.dma_start(qsh[j][:], qT[h, :, j * QC : (j + 1) * QC])
            vsh = []
            for half in range(2):
                row = []
                for j in range(2):
                    t = pv.tile(
                        [128, KB * 65 // 2], f16, tag=f"v{half}{j}", name=f"v{half}{j}"
                    )
                    nc.sync.dma_start(
                        t[:],
                        vp[h, half, :, j * (KB * 65 // 2) : (j + 1) * (KB * 65 // 2)],
                    )
                    row.append(t)
                vsh.append(row)
            return ksh, qsh, vsh

        # PE warm-up: tiny back-to-back matmuls flip the HAM clock gate to
        # 8/8 while the first head's DMAs are in flight.
        wu_w = psingle.tile([128, 128], f16)
        nc.vector.memset(wu_w, 0.0)
        wu_ps = psc.tile([128, 3 * QC], f32, tag="sc0")
        for _ in range(N_WARMUP_MM):
            nc.tensor.matmul(
                wu_ps[:, 0:128], lhsT=wu_w[:], rhs=wu_w[:], start=True, stop=True
            )

        loads = emit_loads(0)

        mask_t = psingle.tile([128, 1], i32)
        nc.vector.memset(mask_t, 0x007FFFFF)
        rprime_t = psingle.tile([128, 1], f32)
        nc.vector.memset(rprime_t, RPRIME)

        for h in range(HPC):
            ksh, qsh, vsh = loads
            if h + 1 < HPC:
                # prefetch the next head's inputs behind this head's compute
                loads = emit_loads(h + 1)


            outc = pout.tile([65, S], f32)
            st = pst.tile([65, N_QC, 6], f32)

            e_tiles = {}  # (qc, g) -> e tile
            accs = {}     # qc -> (a1, a2)

            def emit_A(qc, g):
                """Scores + exp for group g of chunk qc."""
                gs, gl = GROUPS[g]
                L = gl * QC
                last = h == HPC - 1 and qc == N_QC - 1
                dve_groups = (
                    DVE_GROUPS_LAST
                    if last
                    else (DVE_GROUPS_EVEN if qc % 2 == 0 else DVE_GROUPS_ODD)
                )
                sc = psc.tile([128, 3 * QC], f32, tag=f"sc{g % 2}")
                for i in range(gs, gs + gl):
                    kb, half = divmod(i, 2)
                    col = (i - gs) * QC
                    ksk = ksh[kb // 8][:, ts(kb % 8, 128)]
                    nc.tensor.matmul(
                        sc[:, col : col + QC],
                        lhsT=ksk[64 * half : 64 * (half + 1), :],
                        rhs=qsh[qc][64 * half : 64 * (half + 1), :],
                        start=True,
                        stop=True,
                    )
                e = pe.tile([128, 3 * QC], f16, tag="e")
                e_tiles[(qc, g)] = e
                if g in dve_groups:
                    w32 = pw.tile([128, 3 * QC], i32, tag=f"w{g % 2}")
                    nc.vector.tensor_scalar(
                        out=w32[:, 0:L],
                        in0=sc[:, 0:L],
                        scalar1=A2,
                        scalar2=B2,
                        op0=OP.mult,
                        op1=OP.add,
                    )
                    nc.vector._custom_dve(
                        expop,
                        out=e[:, 0:L],
                        in0=w32[:, 0:L].bitcast(f32),
                        in1=rprime_t[:],
                        s0=mask_t[:].bitcast(f32),
                        s1=PPRIME,
                        imm2=QPRIME,
                    )
                else:
                    nc.scalar.activation(e[:, 0:L], sc[:, 0:L], AF.Exp, scale=SCALE)

            def emit_B(qc, g):
                """PV accumulation for group g of chunk qc."""
                gs, gl = GROUPS[g]
                a1, a2 = accs[qc]
                for i in range(gs, gs + gl):
                    kb, half = divmod(i, 2)
                    col = (i - gs) * QC
                    acc = a1 if half == 0 else a2
                    vk = vsh[half][kb // 8][:, ts(kb % 8, 65)]
                    nc.tensor.matmul(
                        acc[:],
                        lhsT=vk,
                        rhs=e_tiles.pop((qc, g))[:, col : col + QC]
                        if i == gs + gl - 1
                        else e_tiles[(qc, g)][:, col : col + QC],
                        start=(kb == 0),
                        stop=(kb == KB - 1),
                    )

            pending_evict = []
            pending_pre = []
            pending = []
            pending_next = []
            for qc in range(N_QC):
                last = h == HPC - 1 and qc == N_QC - 1
                if qc == 0:
                    emit_A(0, 0)
                    emit_A(0, 1)
                for g in range(2, len(GROUPS)):
                    if g == 2:
                        # previous chunk's accumulator evictions slot in here
                        # so they never head-of-line block this chunk's first
                        # activations on the ScalarE queue; the reciprocals
                        # follow and fill the DVE's early idle window
                        for fn in pending_evict:
                            fn()
                        pending_evict = []
                        for fn in pending_pre:
                            fn()
                        pending_pre = []
                    emit_A(qc, g)
                # previous chunk's deferred combine sits behind this chunk's
                # exp stream on the DVE
                for fn in pending:
                    fn()
                pending = pending_next
                pending_next = []

                acc1 = pacc.tile([65, QC], f32, tag="a1")
                acc2 = pacc.tile([65, QC], f32, tag="a2")
                accs[qc] = (acc1, acc2)
                for g in range(0, 9):
                    emit_B(qc, g)
                # software-pipeline the chunk boundary: the next chunk's first
                # score groups go ahead of this chunk's tail PVs in the PE
                # stream so the activation rotation never stalls on them
                if qc + 1 < N_QC:
                    emit_A(qc + 1, 0)
                    emit_A(qc + 1, 1)
                for g in range(9, len(GROUPS)):
                    emit_B(qc, g)

                a1, a2 = accs.pop(qc)
                if last:
                    # banks are free after this — normalize straight out of
                    # PSUM, halves pipelined, to shorten the exposed tail
                    rb1r = pst.tile([1, QC], f32, tag="rb1r")
                    nc.vector.reciprocal_approx_fast(rb1r[:], a1[0:1, :])
                    rb1 = pep.tile([65, QC], f32, tag="rb1")
                    nc.gpsimd.partition_broadcast(rb1[:], rb1r[:], channels=65)
                    rb2r = pst.tile([1, QC], f32, tag="rb2r")
                    nc.vector.reciprocal_approx_fast(rb2r[:], a2[0:1, :])
                    rb2 = pep.tile([65, QC], f32, tag="rb2")
                    nc.gpsimd.partition_broadcast(rb2[:], rb2r[:], channels=65)
                    t1 = pep.tile([65, QC], f32, tag="lt1")
                    nc.vector.tensor_mul(t1[:], a1[:], rb1[:])
                    t2 = pep.tile([65, QC], f32, tag="lt2")
                    nc.vector.tensor_mul(t2[:], a2[:], rb2[:])
                    nc.vector.tensor_sub(outc[:, ts(qc, QC)], t1[:], t2[:])
                    nc.vector.bn_stats(st[:, qc, :], outc[:, ts(qc, QC)])
                    nc.sync.dma_start(outT[h, :, ts(qc, QC)], outc[1:65, ts(qc, QC)])
                else:
                    sa1 = psa.tile([65, QC], f32, tag=f"sa1_{qc % 2}")
                    sa2 = psa.tile([65, QC], f32, tag=f"sa2_{qc % 2}")

                    def _evict(a1=a1, a2=a2, sa1=sa1, sa2=sa2):
                        nc.scalar.copy(sa1[:], a1[:])
                        nc.scalar.copy(sa2[:], a2[:])

                    def _recips(qc=qc, sa1=sa1, sa2=sa2):
                        rb1r = pst.tile([1, QC], f32, tag="rb1r")
                        nc.vector.reciprocal_approx_fast(rb1r[:], sa1[0:1, :])
                        rb1 = pep.tile([65, QC], f32, tag="rb1")
                        nc.gpsimd.partition_broadcast(rb1[:], rb1r[:], channels=65)
                        rb2r = pst.tile([1, QC], f32, tag="rb2r")
                        nc.vector.reciprocal_approx_fast(rb2r[:], sa2[0:1, :])
                        rb2 = pep.tile([65, QC], f32, tag="rb2")
                        nc.gpsimd.partition_broadcast(rb2[:], rb2r[:], channels=65)
                        return rb1, rb2

                    def _combine(rbs, qc=qc, sa1=sa1, sa2=sa2, h=h):
                        rb1, rb2 = rbs
                        t1 = pep.tile([65, QC], f32, tag=f"t1_{qc % 2}")
                        nc.vector.tensor_mul(t1[:], sa1[:], rb1[:])
                        t2 = pep.tile([65, QC], f32, tag=f"t2_{qc % 2}")
                        nc.vector.tensor_mul(t2[:], sa2[:], rb2[:])
                        nc.vector.tensor_sub(outc[:, ts(qc, QC)], t1[:], t2[:])
                        nc.vector.bn_stats(st[:, qc, :], outc[:, ts(qc, QC)])
                        # un-affined diff streams out now; the host applies
                        # the per-head GroupNorm affine during unsharding
                        nc.sync.dma_start(
                            outT[h, :, ts(qc, QC)], outc[1:65, ts(qc, QC)]
                        )

                    pending_evict.append(_evict)
                    rbs = []
                    pending_pre.append(lambda r=rbs, f=_recips: r.append(f()))
                    pending.append(lambda r=rbs, f=_combine: f(r[0]))
            for fn in pending_evict:
                fn()
            pending_evict = []
            for fn in pending_pre:
                fn()
            pending_pre = []
            for fn in pending:
                fn()
            pending = []
            for fn in pending_next:
                fn()
            pending_next = []

            # ---- head finalize: per-partition (mean, var) over S leave the
            # device; the 64-way scalar reduction + rsqrt + affine happen on
            # the host during unsharding ----
            mv = pst.tile([65, 2], f32)
            nc.vector.bn_aggr(mv[:], st[:])
            nc.sync.dma_start(sgtb[h], mv[:])

    nc.compile()
    return nc


def _get_nc():
    if "nc" not in _CACHE:
        _CACHE["nc"] = _build_nc()
    return _CACHE["nc"]


def _host_prep(q, k, v, lq1, lq2, lk1, lk2, gamma, beta):
    """Build per-core input maps."""
    q = np.asarray(q, dtype=np.float32)
    k = np.asarray(k, dtype=np.float32)
    v = np.asarray(v, dtype=np.float32)
    lam = float(
        np.exp(np.float32(np.dot(lq1, lk1)))
        - np.exp(np.float32(np.dot(lq2, lk2)))
        + LAMBDA_INIT
    )
    g2 = (np.asarray(gamma, np.float32) * (1.0 - LAMBDA_INIT)).reshape(H, D)
    b2 = (np.asarray(beta, np.float32) * (1.0 - LAMBDA_INIT)).reshape(H, D)

    in_maps = []
    for c in range(N_CORES):
        heads = range(c * HPC, (c + 1) * HPC)
        qTa = np.empty((HPC, 128, S), np.float16)
        kTa = np.empty((HPC, 128, S), np.float16)
        vpa = np.empty((HPC, 2, 128, KB * 65), np.float16)
        gba = np.empty((HPC, 64, 2), np.float32)
        for i, hh in enumerate(heads):
            qTa[i] = q[0, hh].T.astype(np.float16)
            kTa[i] = k[0, hh].T.astype(np.float16)
            vh = v[0, hh]  # [S, 64]
            v1 = np.concatenate([np.ones((S, 1), np.float32), vh], axis=1)
            v2 = np.concatenate([np.ones((S, 1), np.float32), lam * vh], axis=1)
            # SBUF image: [partition(key within block), kblock*65 + col]
            vpa[i, 0] = (
                v1.reshape(KB, 128, 65).transpose(1, 0, 2).reshape(128, KB * 65)
            ).astype(np.float16)
            vpa[i, 1] = (
                v2.reshape(KB, 128, 65).transpose(1, 0, 2).reshape(128, KB * 65)
            ).astype(np.float16)
            gba[i, :, 0] = g2[hh]
            gba[i, :, 1] = b2[hh]
        in_maps.append({"qT": qTa, "kT": kTa, "vp": vpa, "gb": gba})
    return in_maps


def kernel(q, k, v, lq1, lq2, lk1, lk2, gamma, beta, _trace=False, _tmpdir=None):
    from concourse.bass_utils import run_bass_kernel_spmd

    nc = _get_nc()
    in_maps = _host_prep(q, k, v, lq1, lq2, lk1, lk2, gamma, beta)
    res = run_bass_kernel_spmd(
        nc,
        in_maps,
        core_ids=list(range(N_CORES)),
        trace=_trace,
        tmpdir=_tmpdir,
    )
    g2 = (np.asarray(gamma, np.float32) * (1.0 - LAMBDA_INIT)).reshape(H, D)
    b2 = (np.asarray(beta, np.float32) * (1.0 - LAMBDA_INIT)).reshape(H, D)
    out = np.empty((B, H, S, D), np.float32)
    for c in range(N_CORES):
        outT = res.results[c]["outT"]  # [HPC, 64, S] un-affined diff
        mvs = res.results[c]["sgtb"]   # [HPC, 65, 2] per-partition (mean, var)
        for i in range(HPC):
            hh = c * HPC + i
            mean_p = mvs[i, 1:65, 0].astype(np.float64)
            var_p = mvs[i, 1:65, 1].astype(np.float64)
            mu = mean_p.mean()
            var = (var_p + mean_p**2).mean() - mu * mu
            rstd = 1.0 / np.sqrt(var + EPS)
            sg = (rstd * g2[hh]).astype(np.float32)[:, None]
            tb = (b2[hh] - mu * rstd * g2[hh]).astype(np.float32)[:, None]
            out[0, hh] = (outT[i] * sg + tb).T
    if _trace:
        _CACHE["last_results"] = res
    return out


# revision 58
# speedup vs baseline: 1.0617x; 1.0268x over previous
"""Differential attention (two-softmax diff + GroupNorm) on 8 TRN2 cores.

Sharding: 16 heads / 8 cores = 2 heads per core (head-parallel, no
collectives). GroupNorm stats are per-(batch, head) so each core is fully
independent.

Device layout (host prepares everything):
  - Q, K per head are host-transposed to [128(d), 2048(s)] fp16: partitions
    0-63 hold half-1 (q1/k1), partitions 64-127 hold half-2. QK^T contracts
    over the partition dim, producing transposed score blocks S^T[key, query]
    in PSUM (fp32). The two 64-contraction halves auto-derive PE row-tile
    positions (0,0)/(64,0) and execute concurrently on the array.
  - V per head is prefixed with a ones column (V' = [1 | V], 65 cols, fp16)
    pre-arranged as [128(key-in-block), 16*65]: the PV matmul yields the
    softmax denominator on partition 0 and the numerator on partitions 1-64.
    lam is folded into half-2's V on the host.

ScalarE's exp over 2*S^2 scores/head is the bottleneck engine (~1.1 ns per
lane-element + ~260ns fixed per ACTIVATE). Levers:
  1. Bigger activation batches: scores accumulate in two rotating 3-bank
     PSUM tiles [128, 1536] so each ACTIVATE covers 3 slabs.
  2. ~25% of slab-groups are exp'd on the VectorE via a 2-instruction
     Schraudolph pipeline with cubic mantissa correction (max rel err
     ~7.7e-4, same class as the fp16 store quantization):
       i   = round_f32(s*A2 + B2)      stock tensor_scalar, f32->i32 convert
       w   = bitcast_f32(i)            = 2^(t+delta) * (1+f)/2^f
       m   = (i & 0x7FFFFF) | One.bits custom op: mantissa -> [1,2)
       e   = (((p'-m)m + q')m + r') * w   cubic corr, |c3| folded into B2
     The finisher is one 8-stage custom-DVE op (and, or, sub, mul, add,
     mul, add, mul) registered at import time.

Scheduling: per chunk, pass A emits all QK + exp (e production runs ahead),
pass B the serial PV accumulation; the next chunk's first two score groups
are emitted before this chunk's tail PVs (software-pipelined boundary).
Accumulators are evicted by ScalarE copies slotted into the next chunk's
act stream; the normalize (reciprocal_approx_fast on the den rows, GpSimd
partition broadcasts, multiply/subtract) is deferred behind the next
chunk's exp work on the DVE. bn stats stay on-device per chunk; the final
64-way scalar stat reduction, rsqrt and GroupNorm affine are applied on
the host during unsharding (outT carries the un-affined diff, sgtb the
per-partition (mean, var)).
"""

import math

import numpy as np

B, H, S, D = 1, 16, 2048, 64
N_CORES = 8
HPC = H // N_CORES  # heads per core
QC = 512            # query-chunk width
N_QC = S // QC
KB = S // 128       # key blocks of 128
LAMBDA_INIT = 0.8
EPS = 1e-5
SCALE = 1.0 / math.sqrt(D)
N_WARMUP_MM = 14

# cubic minimax fit of R(m) = 2^(m-1)/m on [1,2):  c3 m^3 + c2 m^2 + c1 m + c0
_C3 = -0.10246085749846692
_C2 = 0.69063801
_C1 = -1.35417106
_C0 = 1.76527539
PPRIME = -_C2 / _C3            # +6.7405058
QPRIME = -_C1 / _C3            # -13.216472
RPRIME = -_C0 / _C3            # +17.228778
DELTA = math.log2(-_C3)        # fold |c3| into the exponent bias
A2 = float(np.float32(math.log2(math.e) * SCALE * 2.0**23))
B2 = float(np.float32((127.0 + DELTA) * 2.0**23))

# slab-groups per 512-query chunk: 32 slabs of [128,512] scores -> 11 groups
GROUPS = [(i * 3, 3) for i in range(10)] + [(30, 2)]
# group indices handled by the VectorE exp pipeline (rest: ScalarE ACTIVATE);
# mid placement keeps the e-latency off both the rotation head and the PV
# chain tail; alternation balances the two engines at ~2.5 groups/chunk
DVE_GROUPS_EVEN = (2, 5, 10)
DVE_GROUPS_ODD = (3, 7, 10)
DVE_GROUPS_LAST = (3, 7)

_CACHE = {}


def _get_exp_op():
    """Register (once) and return the custom-DVE exp-finisher op."""
    if "expop" in _CACHE:
        return _CACHE["expop"]
    from concourse import dve_ops
    from concourse.dve_spec import (
        AluOp,
        Bin,
        C0,
        C1,
        C2,
        C3,
        One,
        Spec,
        Src0,
        _spill_c3_to_src1,
        lower,
    )
    from concourse.dve_uop import DveOpSpec

    for existing in dve_ops.OPS:
        if existing.name == "ANT_EXP2_FINISH":
            _CACHE["expop"] = existing
            return existing

    mm = Bin(AluOp.BITWISE_AND, Src0, C0)
    mo = Bin(AluOp.BITWISE_OR, mm, One)
    t5 = ((C1 - mo) * mo + C2) * mo + C3
    body = _spill_c3_to_src1(t5 * Src0)

    def _ref(in0, in1, s0, s1, imm2):
        bits = np.asarray(in0, np.float32).view(np.int32)
        s0i = np.asarray(s0).view(np.int32) if isinstance(s0, np.ndarray) else np.int32(s0)
        m = ((bits & s0i) | np.int32(0x3F800000)).view(np.float32)
        t = ((np.float32(s1) - m) * m + np.float32(imm2)) * m + np.asarray(
            in1, np.float32
        )
        return t * np.asarray(in0, np.float32)

    spec = Spec(body=body, reference=_ref)
    op = dve_ops.DveOp("ANT_EXP2_FINISH", spec, subdim=False, uops_sha={})
    dve_ops.OPS.append(op)
    dve_ops._SUB_OPCODE_FOR_NAME[op.name] = dve_ops._CUSTOM_DVE_ROW_BASE + len(
        dve_ops.OPS
    ) - 1
    dve_ops.CUSTOM_DVE_SPECS[op.name] = spec
    for ver in ("v3", "v4"):
        tmp = DveOpSpec(
            name=op.name,
            opcode=dve_ops.get_dve_sub_opcode(op.name),
            uops=lower(spec, ver=ver),
            rd1_en=True,
        )
        op.uops_sha[ver] = tmp.sha(ver)
    _CACHE["expop"] = op
    return op


def _build_nc():
    from contextlib import ExitStack

    import concourse.bacc as bacc
    import concourse.bass as bass
    import concourse.tile as tile
    from concourse import bass_isa, mybir

    f32 = mybir.dt.float32
    f16 = mybir.dt.float16
    i32 = mybir.dt.int32
    AF = mybir.ActivationFunctionType
    OP = mybir.AluOpType
    ts = bass.ts

    expop = _get_exp_op()

    nc = bacc.Bacc("TRN2", target_bir_lowering=False, debug=False)

    qT = nc.dram_tensor("qT", [HPC, 128, S], f16, kind="ExternalInput").ap()
    kT = nc.dram_tensor("kT", [HPC, 128, S], f16, kind="ExternalInput").ap()
    vp = nc.dram_tensor("vp", [HPC, 2, 128, KB * 65], f16, kind="ExternalInput").ap()
    gb = nc.dram_tensor("gb", [HPC, 64, 2], f32, kind="ExternalInput").ap()
    outT = nc.dram_tensor("outT", [HPC, 64, S], f32, kind="ExternalOutput").ap()
    sgtb = nc.dram_tensor("sgtb", [HPC, 65, 2], f32, kind="ExternalOutput").ap()

    with tile.TileContext(nc) as tc, ExitStack() as ctx:
        pq = ctx.enter_context(tc.tile_pool(name="pq", bufs=2))
        pk = ctx.enter_context(tc.tile_pool(name="pk", bufs=2))
        pv = ctx.enter_context(tc.tile_pool(name="pv", bufs=2))
        pe = ctx.enter_context(tc.tile_pool(name="pe", bufs=13))
        pw = ctx.enter_context(tc.tile_pool(name="pw", bufs=1))
        pep = ctx.enter_context(tc.tile_pool(name="pep", bufs=2))
        psa = ctx.enter_context(tc.tile_pool(name="psa", bufs=2))
        pout = ctx.enter_context(tc.tile_pool(name="pout", bufs=2))
        pst = ctx.enter_context(tc.tile_pool(name="pst", bufs=2))
        psingle = ctx.enter_context(tc.tile_pool(name="psingle", bufs=1))
        psc = ctx.enter_context(tc.tile_pool(name="psc", bufs=1, space="PSUM"))
        pacc = ctx.enter_context(tc.tile_pool(name="pacc", bufs=1, space="PSUM"))

        def emit_loads(h):
            """DMA the head's inputs; split so the first matmuls start early."""
            ksh = []
            for j in range(2):
                ks_t = pk.tile([128, S // 2], f16, tag=f"ks{j}", name=f"ks{j}")
                ksh.append(ks_t)
            qsh = []
            for j in range(N_QC):
                qs_t = pq.tile([128, QC], f16, tag=f"qs{j}", name=f"qs{j}")
                qsh.append(qs_t)
            nc.sync.dma_start(ksh[0][:], kT[h, :, 0 : S // 2])
            nc.sync.dma_start(qsh[0][:], qT[h, :, 0:QC])
            nc.sync.dma_start(ksh[1][:], kT[h, :, S // 2 : S])
            for j in range(1, N_QC):
                nc.sync.dma_start(qsh[j][:], qT[h, :, j * QC : (j + 1) * QC])
            vsh = []
            for half in range(2):
                row = []
                for j in range(2):
                    t = pv.tile(
                        [128, KB * 65 // 2], f16, tag=f"v{half}{j}", name=f"v{half}{j}"
                    )
                    nc.sync.dma_start(
                        t[:],
                        vp[h, half, :, j * (KB * 65 // 2) : (j + 1) * (KB * 65 // 2)],
                    )
                    row.append(t)
                vsh.append(row)
            return ksh, qsh, vsh

        # PE warm-up: tiny back-to-back matmuls flip the HAM clock gate to
        # 8/8 while the first head's DMAs are in flight.
        wu_w = psingle.tile([128, 128], f16)
        nc.vector.memset(wu_w, 0.0)
        wu_ps = psc.tile([128, 3 * QC], f32, tag="sc0")
        for _ in range(N_WARMUP_MM):
            nc.tensor.matmul(
                wu_ps[:, 0:128], lhsT=wu_w[:], rhs=wu_w[:], start=True, stop=True
            )

        loads = emit_loads(0)

        mask_t = psingle.tile([128, 1], i32)
        nc.vector.memset(mask_t, 0x007FFFFF)
        rprime_t = psingle.tile([128, 1], f32)
        nc.vector.memset(rprime_t, RPRIME)

        for h in range(HPC):
            ksh, qsh, vsh = loads
            if h + 1 < HPC:
                # prefetch the next head's inputs behind this head's compute
                loads = emit_loads(h + 1)


            outc = pout.tile([65, S], f32)
            st = pst.tile([65, N_QC, 6], f32)

            e_tiles = {}  # (qc, g) -> e tile
            accs = {}     # qc -> (a1, a2)

            def emit_A(qc, g):
                """Scores + exp for group g of chunk qc."""
                gs, gl = GROUPS[g]
                L = gl * QC
                last = h == HPC - 1 and qc == N_QC - 1
                dve_groups = (
                    DVE_GROUPS_LAST
                    if last
                    else (DVE_GROUPS_EVEN if qc % 2 == 0 else DVE_GROUPS_ODD)
                )
                sc = psc.tile([128, 3 * QC], f32, tag=f"sc{g % 2}")
                for i in range(gs, gs + gl):
                    kb, half = divmod(i, 2)
                    col = (i - gs) * QC
                    ksk = ksh[kb // 8][:, ts(kb % 8, 128)]
                    nc.tensor.matmul(
                        sc[:, col : col + QC],
                        lhsT=ksk[64 * half : 64 * (half + 1), :],
                        rhs=qsh[qc][64 * half : 64 * (half + 1), :],
                        start=True,
                        stop=True,
                    )
                e = pe.tile([128, 3 * QC], f16, tag="e")
                e_tiles[(qc, g)] = e
                if g in dve_groups:
                    w32 = pw.tile([128, 3 * QC], i32, tag=f"w{g % 2}")
                    nc.vector.tensor_scalar(
                        out=w32[:, 0:L],
                        in0=sc[:, 0:L],
                        scalar1=A2,
                        scalar2=B2,
                        op0=OP.mult,
                        op1=OP.add,
                    )
                    nc.vector._custom_dve(
                        expop,
                        out=e[:, 0:L],
                        in0=w32[:, 0:L].bitcast(f32),
                        in1=rprime_t[:],
                        s0=mask_t[:].bitcast(f32),
                        s1=PPRIME,
                        imm2=QPRIME,
                    )
                else:
                    nc.scalar.activation(e[:, 0:L], sc[:, 0:L], AF.Exp, scale=SCALE)

            def emit_B(qc, g):
                """PV accumulation for group g of chunk qc."""
                gs, gl = GROUPS[g]
                a1, a2 = accs[qc]
                for i in range(gs, gs + gl):
                    kb, half = divmod(i, 2)
                    col = (i - gs) * QC
                    acc = a1 if half == 0 else a2
                    vk = vsh[half][kb // 8][:, ts(kb % 8, 65)]
                    nc.tensor.matmul(
                        acc[:],
                        lhsT=vk,
                        rhs=e_tiles.pop((qc, g))[:, col : col + QC]
                        if i == gs + gl - 1
                        else e_tiles[(qc, g)][:, col : col + QC],
                        start=(kb == 0),
                        stop=(kb == KB - 1),
                    )

            pending_evict = []
            pending_pre = []
            pending = []
            pending_next = []
            for qc in range(N_QC):
                last = h == HPC - 1 and qc == N_QC - 1
                if qc == 0:
                    emit_A(0, 0)
                    emit_A(0, 1)
                for g in range(2, len(GROUPS)):
                    if g == 2:
                        # previous chunk's accumulator evictions slot in here
                        # so they never head-of-line block this chunk's first
                        # activations on the ScalarE queue; the reciprocals
                        # follow and fill the DVE's early idle window
                        for fn in pending_evict:
                            fn()
                        pending_evict = []
                        for fn in pending_pre:
                            fn()
                        pending_pre = []
                    emit_A(qc, g)
                # previous chunk's deferred combine sits behind this chunk's
                # exp stream on the DVE
                for fn in pending:
                    fn()
                pending = pending_next
                pending_next = []

                acc1 = pacc.tile([65, QC], f32, tag="a1")
                acc2 = pacc.tile([65, QC], f32, tag="a2")
                accs[qc] = (acc1, acc2)
                for g in range(0, 9):
                    emit_B(qc, g)
                # software-pipeline the chunk boundary: the next chunk's first
                # score groups go ahead of this chunk's tail PVs in the PE
                # stream so the activation rotation never stalls on them
                if qc + 1 < N_QC:
                    emit_A(qc + 1, 0)
                    emit_A(qc + 1, 1)
                for g in range(9, len(GROUPS)):
                    emit_B(qc, g)

                a1, a2 = accs.pop(qc)
                if last:
                    # banks are free after this — normalize straight out of
                    # PSUM; one merged broadcast avoids the GpSimd pipe
                    # drain between two back-to-back broadcasts
                    rbr = pst.tile([1, 2 * QC], f32, tag="rbr")
                    nc.vector.reciprocal_approx_fast(rbr[:, 0:QC], a1[0:1, :])
                    nc.vector.reciprocal_approx_fast(rbr[:, QC:], a2[0:1, :])
                    rbw = pep.tile([65, 2 * QC], f32, tag="lrb")
                    nc.gpsimd.partition_broadcast(rbw[:], rbr[:], channels=65)
                    t1 = pep.tile([65, QC], f32, tag="lt1")
                    nc.vector.tensor_mul(t1[:], a1[:], rbw[:, 0:QC])
                    t2 = pep.tile([65, QC], f32, tag="lt2")
                    nc.vector.tensor_mul(t2[:], a2[:], rbw[:, QC:])
                    nc.vector.tensor_sub(outc[:, ts(qc, QC)], t1[:], t2[:])
                    nc.vector.bn_stats(st[:, qc, :], outc[:, ts(qc, QC)])
                    nc.sync.dma_start(outT[h, :, ts(qc, QC)], outc[1:65, ts(qc, QC)])
                else:
                    sa1 = psa.tile([65, QC], f32, tag=f"sa1_{qc % 2}")
                    sa2 = psa.tile([65, QC], f32, tag=f"sa2_{qc % 2}")

                    def _evict(a1=a1, a2=a2, sa1=sa1, sa2=sa2):
                        nc.scalar.copy(sa1[:], a1[:])
                        nc.scalar.copy(sa2[:], a2[:])

                    def _recips(qc=qc, sa1=sa1, sa2=sa2):
                        rb1r = pst.tile([1, QC], f32, tag="rb1r")
                        nc.vector.reciprocal_approx_fast(rb1r[:], sa1[0:1, :])
                        rb1 = pep.tile([65, QC], f32, tag="rb1")
                        nc.gpsimd.partition_broadcast(rb1[:], rb1r[:], channels=65)
                        rb2r = pst.tile([1, QC], f32, tag="rb2r")
                        nc.vector.reciprocal_approx_fast(rb2r[:], sa2[0:1, :])
                        rb2 = pep.tile([65, QC], f32, tag="rb2")
                        nc.gpsimd.partition_broadcast(rb2[:], rb2r[:], channels=65)
                        return rb1, rb2

                    def _combine(rbs, qc=qc, sa1=sa1, sa2=sa2, h=h):
                        rb1, rb2 = rbs
                        t1 = pep.tile([65, QC], f32, tag=f"t1_{qc % 2}")
                        nc.vector.tensor_mul(t1[:], sa1[:], rb1[:])
                        t2 = pep.tile([65, QC], f32, tag=f"t2_{qc % 2}")
                        nc.vector.tensor_mul(t2[:], sa2[:], rb2[:])
                        nc.vector.tensor_sub(outc[:, ts(qc, QC)], t1[:], t2[:])
                        nc.vector.bn_stats(st[:, qc, :], outc[:, ts(qc, QC)])
                        # un-affined diff streams out now; the host applies
                        # the per-head GroupNorm affine during unsharding
                        nc.sync.dma_start(
                            outT[h, :, ts(qc, QC)], outc[1:65, ts(qc, QC)]
                        )

                    pending_evict.append(_evict)
                    rbs = []
                    pending_pre.append(lambda r=rbs, f=_recips: r.append(f()))
                    pending.append(lambda r=rbs, f=_combine: f(r[0]))
            for fn in pending_evict:
                fn()
            pending_evict = []
            for fn in pending_pre:
                fn()
            pending_pre = []
            for fn in pending:
                fn()
            pending = []
            for fn in pending_next:
                fn()
            pending_next = []

            # ---- head finalize: per-partition (mean, var) over S leave the
            # device; the 64-way scalar reduction + rsqrt + affine happen on
            # the host during unsharding ----
            mv = pst.tile([65, 2], f32)
            nc.vector.bn_aggr(mv[:], st[:])
            nc.sync.dma_start(sgtb[h], mv[:])

    nc.compile()
    return nc


def _get_nc():
    if "nc" not in _CACHE:
        _CACHE["nc"] = _build_nc()
    return _CACHE["nc"]


def _host_prep(q, k, v, lq1, lq2, lk1, lk2, gamma, beta):
    """Build per-core input maps."""
    q = np.asarray(q, dtype=np.float32)
    k = np.asarray(k, dtype=np.float32)
    v = np.asarray(v, dtype=np.float32)
    lam = float(
        np.exp(np.float32(np.dot(lq1, lk1)))
        - np.exp(np.float32(np.dot(lq2, lk2)))
        + LAMBDA_INIT
    )
    g2 = (np.asarray(gamma, np.float32) * (1.0 - LAMBDA_INIT)).reshape(H, D)
    b2 = (np.asarray(beta, np.float32) * (1.0 - LAMBDA_INIT)).reshape(H, D)

    in_maps = []
    for c in range(N_CORES):
        heads = range(c * HPC, (c + 1) * HPC)
        qTa = np.empty((HPC, 128, S), np.float16)
        kTa = np.empty((HPC, 128, S), np.float16)
        vpa = np.empty((HPC, 2, 128, KB * 65), np.float16)
        gba = np.empty((HPC, 64, 2), np.float32)
        for i, hh in enumerate(heads):
            qTa[i] = q[0, hh].T.astype(np.float16)
            kTa[i] = k[0, hh].T.astype(np.float16)
            vh = v[0, hh]  # [S, 64]
            v1 = np.concatenate([np.ones((S, 1), np.float32), vh], axis=1)
            v2 = np.concatenate([np.ones((S, 1), np.float32), lam * vh], axis=1)
            # SBUF image: [partition(key within block), kblock*65 + col]
            vpa[i, 0] = (
                v1.reshape(KB, 128, 65).transpose(1, 0, 2).reshape(128, KB * 65)
            ).astype(np.float16)
            vpa[i, 1] = (
                v2.reshape(KB, 128, 65).transpose(1, 0, 2).reshape(128, KB * 65)
            ).astype(np.float16)
            gba[i, :, 0] = g2[hh]
            gba[i, :, 1] = b2[hh]
        in_maps.append({"qT": qTa, "kT": kTa, "vp": vpa, "gb": gba})
    return in_maps


def kernel(q, k, v, lq1, lq2, lk1, lk2, gamma, beta, _trace=False, _tmpdir=None):
    from concourse.bass_utils import run_bass_kernel_spmd

    nc = _get_nc()
    in_maps = _host_prep(q, k, v, lq1, lq2, lk1, lk2, gamma, beta)
    res = run_bass_kernel_spmd(
        nc,
        in_maps,
        core_ids=list(range(N_CORES)),
        trace=_trace,
        tmpdir=_tmpdir,
    )
    g2 = (np.asarray(gamma, np.float32) * (1.0 - LAMBDA_INIT)).reshape(H, D)
    b2 = (np.asarray(beta, np.float32) * (1.0 - LAMBDA_INIT)).reshape(H, D)
    out = np.empty((B, H, S, D), np.float32)
    for c in range(N_CORES):
        outT = res.results[c]["outT"]  # [HPC, 64, S] un-affined diff
        mvs = res.results[c]["sgtb"]   # [HPC, 65, 2] per-partition (mean, var)
        for i in range(HPC):
            hh = c * HPC + i
            mean_p = mvs[i, 1:65, 0].astype(np.float64)
            var_p = mvs[i, 1:65, 1].astype(np.float64)
            mu = mean_p.mean()
            var = (var_p + mean_p**2).mean() - mu * mu
            rstd = 1.0 / np.sqrt(var + EPS)
            sg = (rstd * g2[hh]).astype(np.float32)[:, None]
            tb = (b2[hh] - mu * rstd * g2[hh]).astype(np.float32)[:, None]
            out[0, hh] = (outT[i] * sg + tb).T
    if _trace:
        _CACHE["last_results"] = res
    return out


# revision 59
# speedup vs baseline: 1.0768x; 1.0143x over previous
"""Differential attention (two-softmax diff + GroupNorm) on 8 TRN2 cores.

Sharding: 16 heads / 8 cores = 2 heads per core (head-parallel, no
collectives). GroupNorm stats are per-(batch, head) so each core is fully
independent.

Device layout (host prepares everything):
  - Q, K per head are host-transposed to [128(d), 2048(s)] fp16: partitions
    0-63 hold half-1 (q1/k1), partitions 64-127 hold half-2. QK^T contracts
    over the partition dim, producing transposed score blocks S^T[key, query]
    in PSUM (fp32). The two 64-contraction halves auto-derive PE row-tile
    positions (0,0)/(64,0) and execute concurrently on the array.
  - V per head is prefixed with a ones column (V' = [1 | V], 65 cols, fp16)
    pre-arranged as [128(key-in-block), 16*65]: the PV matmul yields the
    softmax denominator on partition 0 and the numerator on partitions 1-64.
    lam is folded into half-2's V on the host.

ScalarE's exp over 2*S^2 scores/head is the bottleneck engine (~1.1 ns per
lane-element + ~260ns fixed per ACTIVATE). Levers:
  1. Bigger activation batches: scores accumulate in two rotating 3-bank
     PSUM tiles [128, 1536] so each ACTIVATE covers 3 slabs.
  2. ~25% of slab-groups are exp'd on the VectorE via a 2-instruction
     Schraudolph pipeline with cubic mantissa correction (max rel err
     ~7.7e-4, same class as the fp16 store quantization):
       i   = round_f32(s*A2 + B2)      stock tensor_scalar, f32->i32 convert
       w   = bitcast_f32(i)            = 2^(t+delta) * (1+f)/2^f
       m   = (i & 0x7FFFFF) | One.bits custom op: mantissa -> [1,2)
       e   = (((p'-m)m + q')m + r') * w   cubic corr, |c3| folded into B2
     The finisher is one 8-stage custom-DVE op (and, or, sub, mul, add,
     mul, add, mul) registered at import time.

Scheduling: per chunk, pass A emits all QK + exp (e production runs ahead),
pass B the serial PV accumulation; the next chunk's first two score groups
are emitted before this chunk's tail PVs (software-pipelined boundary).
Accumulators are evicted by ScalarE copies slotted into the next chunk's
act stream; the normalize (reciprocal_approx_fast on the den rows, GpSimd
partition broadcasts, multiply/subtract) is deferred behind the next
chunk's exp work on the DVE. bn stats stay on-device per chunk; the final
64-way scalar stat reduction, rsqrt and GroupNorm affine are applied on
the host during unsharding (outT carries the un-affined diff, sgtb the
per-partition (mean, var)).
"""

import math

import numpy as np

B, H, S, D = 1, 16, 2048, 64
N_CORES = 8
HPC = H // N_CORES  # heads per core
QC = 512            # query-chunk width
N_QC = S // QC
KB = S // 128       # key blocks of 128
LAMBDA_INIT = 0.8
EPS = 1e-5
SCALE = 1.0 / math.sqrt(D)
N_WARMUP_MM = 14

# cubic minimax fit of R(m) = 2^(m-1)/m on [1,2):  c3 m^3 + c2 m^2 + c1 m + c0
_C3 = -0.10246085749846692
_C2 = 0.69063801
_C1 = -1.35417106
_C0 = 1.76527539
PPRIME = -_C2 / _C3            # +6.7405058
QPRIME = -_C1 / _C3            # -13.216472
RPRIME = -_C0 / _C3            # +17.228778
DELTA = math.log2(-_C3)        # fold |c3| into the exponent bias
A2 = float(np.float32(math.log2(math.e) * SCALE * 2.0**23))
B2 = float(np.float32((127.0 + DELTA) * 2.0**23))

# slab-groups per 512-query chunk: 32 slabs of [128,512] scores -> 11 groups
GROUPS = [(i * 3, 3) for i in range(10)] + [(30, 2)]
# group indices handled by the VectorE exp pipeline (rest: ScalarE ACTIVATE);
# mid placement keeps the e-latency off both the rotation head and the PV
# chain tail; alternation balances the two engines at ~2.5 groups/chunk
DVE_GROUPS_EVEN = (2, 5, 10)
DVE_GROUPS_ODD = (3, 7, 10)
DVE_GROUPS_LAST = (3, 7)

_CACHE = {}


def _get_exp_op():
    """Register (once) and return the custom-DVE exp-finisher op."""
    if "expop" in _CACHE:
        return _CACHE["expop"]
    from concourse import dve_ops
    from concourse.dve_spec import (
        AluOp,
        Bin,
        C0,
        C1,
        C2,
        C3,
        One,
        Spec,
        Src0,
        _spill_c3_to_src1,
        lower,
    )
    from concourse.dve_uop import DveOpSpec

    for existing in dve_ops.OPS:
        if existing.name == "ANT_EXP2_FINISH":
            _CACHE["expop"] = existing
            return existing

    mm = Bin(AluOp.BITWISE_AND, Src0, C0)
    mo = Bin(AluOp.BITWISE_OR, mm, One)
    t5 = ((C1 - mo) * mo + C2) * mo + C3
    body = _spill_c3_to_src1(t5 * Src0)

    def _ref(in0, in1, s0, s1, imm2):
        bits = np.asarray(in0, np.float32).view(np.int32)
        s0i = np.asarray(s0).view(np.int32) if isinstance(s0, np.ndarray) else np.int32(s0)
        m = ((bits & s0i) | np.int32(0x3F800000)).view(np.float32)
        t = ((np.float32(s1) - m) * m + np.float32(imm2)) * m + np.asarray(
            in1, np.float32
        )
        return t * np.asarray(in0, np.float32)

    spec = Spec(body=body, reference=_ref)
    op = dve_ops.DveOp("ANT_EXP2_FINISH", spec, subdim=False, uops_sha={})
    dve_ops.OPS.append(op)
    dve_ops._SUB_OPCODE_FOR_NAME[op.name] = dve_ops._CUSTOM_DVE_ROW_BASE + len(
        dve_ops.OPS
    ) - 1
    dve_ops.CUSTOM_DVE_SPECS[op.name] = spec
    for ver in ("v3", "v4"):
        tmp = DveOpSpec(
            name=op.name,
            opcode=dve_ops.get_dve_sub_opcode(op.name),
            uops=lower(spec, ver=ver),
            rd1_en=True,
        )
        op.uops_sha[ver] = tmp.sha(ver)
    _CACHE["expop"] = op
    return op


def _build_nc():
    from contextlib import ExitStack

    import concourse.bacc as bacc
    import concourse.bass as bass
    import concourse.tile as tile
    from concourse import bass_isa, mybir

    f32 = mybir.dt.float32
    f16 = mybir.dt.float16
    i32 = mybir.dt.int32
    AF = mybir.ActivationFunctionType
    OP = mybir.AluOpType
    ts = bass.ts

    expop = _get_exp_op()

    nc = bacc.Bacc("TRN2", target_bir_lowering=False, debug=False)

    qT = nc.dram_tensor("qT", [HPC, 128, S], f16, kind="ExternalInput").ap()
    kT = nc.dram_tensor("kT", [HPC, 128, S], f16, kind="ExternalInput").ap()
    vp = nc.dram_tensor("vp", [HPC, 2, 128, KB * 65], f16, kind="ExternalInput").ap()
    gb = nc.dram_tensor("gb", [HPC, 64, 2], f32, kind="ExternalInput").ap()
    outT = nc.dram_tensor("outT", [HPC, 64, S], f32, kind="ExternalOutput").ap()
    sgtb = nc.dram_tensor("sgtb", [HPC, 65, 2], f32, kind="ExternalOutput").ap()

    with tile.TileContext(nc) as tc, ExitStack() as ctx:
        pq = ctx.enter_context(tc.tile_pool(name="pq", bufs=2))
        pk = ctx.enter_context(tc.tile_pool(name="pk", bufs=2))
        pv = ctx.enter_context(tc.tile_pool(name="pv", bufs=2))
        pe = ctx.enter_context(tc.tile_pool(name="pe", bufs=13))
        pw = ctx.enter_context(tc.tile_pool(name="pw", bufs=1))
        pep = ctx.enter_context(tc.tile_pool(name="pep", bufs=2))
        psa = ctx.enter_context(tc.tile_pool(name="psa", bufs=2))
        pout = ctx.enter_context(tc.tile_pool(name="pout", bufs=2))
        pst = ctx.enter_context(tc.tile_pool(name="pst", bufs=2))
        psingle = ctx.enter_context(tc.tile_pool(name="psingle", bufs=1))
        psc = ctx.enter_context(tc.tile_pool(name="psc", bufs=1, space="PSUM"))
        pacc = ctx.enter_context(tc.tile_pool(name="pacc", bufs=1, space="PSUM"))

        def emit_loads(h):
            """DMA the head's inputs; split so the first matmuls start early."""
            ksh = []
            for j in range(2):
                ks_t = pk.tile([128, S // 2], f16, tag=f"ks{j}", name=f"ks{j}")
                ksh.append(ks_t)
            qsh = []
            for j in range(N_QC):
                qs_t = pq.tile([128, QC], f16, tag=f"qs{j}", name=f"qs{j}")
                qsh.append(qs_t)
            nc.sync.dma_start(ksh[0][:], kT[h, :, 0 : S // 2])
            nc.sync.dma_start(qsh[0][:], qT[h, :, 0:QC])
            nc.sync.dma_start(ksh[1][:], kT[h, :, S // 2 : S])
            for j in range(1, N_QC):
                nc.sync.dma_start(qsh[j][:], qT[h, :, j * QC : (j + 1) * QC])
            vsh = []
            for half in range(2):
                row = []
                for j in range(2):
                    t = pv.tile(
                        [128, KB * 65 // 2], f16, tag=f"v{half}{j}", name=f"v{half}{j}"
                    )
                    nc.sync.dma_start(
                        t[:],
                        vp[h, half, :, j * (KB * 65 // 2) : (j + 1) * (KB * 65 // 2)],
                    )
                    row.append(t)
                vsh.append(row)
            return ksh, qsh, vsh

        # PE warm-up: tiny back-to-back matmuls flip the HAM clock gate to
        # 8/8 while the first head's DMAs are in flight.
        wu_w = psingle.tile([128, 128], f16)
        nc.vector.memset(wu_w, 0.0)
        wu_ps = psc.tile([128, 3 * QC], f32, tag="sc0")
        for _ in range(N_WARMUP_MM):
            nc.tensor.matmul(
                wu_ps[:, 0:128], lhsT=wu_w[:], rhs=wu_w[:], start=True, stop=True
            )

        loads = emit_loads(0)

        mask_t = psingle.tile([128, 1], i32)
        nc.vector.memset(mask_t, 0x007FFFFF)
        rprime_t = psingle.tile([128, 1], f32)
        nc.vector.memset(rprime_t, RPRIME)

        for h in range(HPC):
            ksh, qsh, vsh = loads
            if h + 1 < HPC:
                # prefetch the next head's inputs behind this head's compute
                loads = emit_loads(h + 1)


            outc = pout.tile([65, S], f32)
            st = pst.tile([65, N_QC, 6], f32)

            e_tiles = {}  # (qc, g) -> e tile
            accs = {}     # qc -> (a1, a2)

            def emit_A(qc, g):
                """Scores + exp for group g of chunk qc."""
                gs, gl = GROUPS[g]
                L = gl * QC
                last = h == HPC - 1 and qc == N_QC - 1
                dve_groups = (
                    DVE_GROUPS_LAST
                    if last
                    else (DVE_GROUPS_EVEN if qc % 2 == 0 else DVE_GROUPS_ODD)
                )
                sc = psc.tile([128, 3 * QC], f32, tag=f"sc{g % 2}")
                for i in range(gs, gs + gl):
                    kb, half = divmod(i, 2)
                    col = (i - gs) * QC
                    ksk = ksh[kb // 8][:, ts(kb % 8, 128)]
                    nc.tensor.matmul(
                        sc[:, col : col + QC],
                        lhsT=ksk[64 * half : 64 * (half + 1), :],
                        rhs=qsh[qc][64 * half : 64 * (half + 1), :],
                        start=True,
                        stop=True,
                    )
                e = pe.tile([128, 3 * QC], f16, tag="e")
                e_tiles[(qc, g)] = e
                if g in dve_groups:
                    w32 = pw.tile([128, 3 * QC], i32, tag=f"w{g % 2}")
                    nc.vector.tensor_scalar(
                        out=w32[:, 0:L],
                        in0=sc[:, 0:L],
                        scalar1=A2,
                        scalar2=B2,
                        op0=OP.mult,
                        op1=OP.add,
                    )
                    nc.vector._custom_dve(
                        expop,
                        out=e[:, 0:L],
                        in0=w32[:, 0:L].bitcast(f32),
                        in1=rprime_t[:],
                        s0=mask_t[:].bitcast(f32),
                        s1=PPRIME,
                        imm2=QPRIME,
                    )
                else:
                    nc.scalar.activation(e[:, 0:L], sc[:, 0:L], AF.Exp, scale=SCALE)

            def emit_B(qc, g):
                """PV accumulation for group g of chunk qc."""
                gs, gl = GROUPS[g]
                a1, a2 = accs[qc]
                for i in range(gs, gs + gl):
                    kb, half = divmod(i, 2)
                    col = (i - gs) * QC
                    acc = a1 if half == 0 else a2
                    vk = vsh[half][kb // 8][:, ts(kb % 8, 65)]
                    nc.tensor.matmul(
                        acc[:],
                        lhsT=vk,
                        rhs=e_tiles.pop((qc, g))[:, col : col + QC]
                        if i == gs + gl - 1
                        else e_tiles[(qc, g)][:, col : col + QC],
                        start=(kb == 0),
                        stop=(kb == KB - 1),
                    )

            pending_evict = []
            pending_pre = []
            pending = []
            pending_next = []
            for qc in range(N_QC):
                last = h == HPC - 1 and qc == N_QC - 1
                if qc == 0:
                    emit_A(0, 0)
                    emit_A(0, 1)
                for g in range(2, len(GROUPS)):
                    if g == 2:
                        # previous chunk's accumulator evictions slot in here
                        # so they never head-of-line block this chunk's first
                        # activations on the ScalarE queue; the reciprocals
                        # follow and fill the DVE's early idle window
                        for fn in pending_evict:
                            fn()
                        pending_evict = []
                        for fn in pending_pre:
                            fn()
                        pending_pre = []
                    emit_A(qc, g)
                # previous chunk's deferred combine sits behind this chunk's
                # exp stream on the DVE
                for fn in pending:
                    fn()
                pending = pending_next
                pending_next = []

                acc1 = pacc.tile([65, QC], f32, tag="a1")
                acc2 = pacc.tile([65, QC], f32, tag="a2")
                accs[qc] = (acc1, acc2)
                for g in range(0, 9):
                    emit_B(qc, g)
                # software-pipeline the chunk boundary: the next chunk's first
                # score groups go ahead of this chunk's tail PVs in the PE
                # stream so the activation rotation never stalls on them
                if qc + 1 < N_QC:
                    emit_A(qc + 1, 0)
                    emit_A(qc + 1, 1)
                for g in range(9, len(GROUPS)):
                    emit_B(qc, g)

                a1, a2 = accs.pop(qc)
                if last:
                    # banks are free after this — normalize straight out of
                    # PSUM, halves pipelined, to shorten the exposed tail
                    rb1r = pst.tile([1, QC], f32, tag="rb1r")
                    nc.vector.reciprocal_approx_fast(rb1r[:], a1[0:1, :])
                    rb1 = pep.tile([65, QC], f32, tag="rb1")
                    nc.gpsimd.partition_broadcast(rb1[:], rb1r[:], channels=65)
                    rb2r = pst.tile([1, QC], f32, tag="rb2r")
                    nc.vector.reciprocal_approx_fast(rb2r[:], a2[0:1, :])
                    rb2 = pep.tile([65, QC], f32, tag="rb2")
                    nc.gpsimd.partition_broadcast(rb2[:], rb2r[:], channels=65)
                    t1 = pep.tile([65, QC], f32, tag="lt1")
                    nc.vector.tensor_mul(t1[:], a1[:], rb1[:])
                    t2 = pep.tile([65, QC], f32, tag="lt2")
                    nc.vector.tensor_mul(t2[:], a2[:], rb2[:])
                    nc.vector.tensor_sub(outc[:, ts(qc, QC)], t1[:], t2[:])
                    nc.vector.bn_stats(st[:, qc, :], outc[:, ts(qc, QC)])
                    nc.sync.dma_start(outT[h, :, ts(qc, QC)], outc[1:65, ts(qc, QC)])
                else:
                    sa1 = psa.tile([65, QC], f32, tag=f"sa1_{qc % 2}")
                    sa2 = psa.tile([65, QC], f32, tag=f"sa2_{qc % 2}")

                    def _evict(a1=a1, a2=a2, sa1=sa1, sa2=sa2):
                        nc.scalar.copy(sa1[:], a1[:])
                        nc.scalar.copy(sa2[:], a2[:])

                    def _recips(qc=qc, sa1=sa1, sa2=sa2):
                        rb1r = pst.tile([1, QC], f32, tag="rb1r")
                        nc.vector.reciprocal_approx_fast(rb1r[:], sa1[0:1, :])
                        rb1 = pep.tile([65, QC], f32, tag="rb1")
                        nc.gpsimd.partition_broadcast(rb1[:], rb1r[:], channels=65)
                        rb2r = pst.tile([1, QC], f32, tag="rb2r")
                        nc.vector.reciprocal_approx_fast(rb2r[:], sa2[0:1, :])
                        rb2 = pep.tile([65, QC], f32, tag="rb2")
                        nc.gpsimd.partition_broadcast(rb2[:], rb2r[:], channels=65)
                        return rb1, rb2

                    def _combine(rbs, qc=qc, sa1=sa1, sa2=sa2, h=h):
                        rb1, rb2 = rbs
                        t1 = pep.tile([65, QC], f32, tag=f"t1_{qc % 2}")
                        nc.vector.tensor_mul(t1[:], sa1[:], rb1[:])
                        t2 = pep.tile([65, QC], f32, tag=f"t2_{qc % 2}")
                        nc.vector.tensor_mul(t2[:], sa2[:], rb2[:])
                        nc.vector.tensor_sub(outc[:, ts(qc, QC)], t1[:], t2[:])
                        nc.vector.bn_stats(st[:, qc, :], outc[:, ts(qc, QC)])
                        # un-affined diff streams out now; the host applies
                        # the per-head GroupNorm affine during unsharding
                        nc.sync.dma_start(
                            outT[h, :, ts(qc, QC)], outc[1:65, ts(qc, QC)]
                        )

                    pending_evict.append(_evict)
                    rbs = []
                    pending_pre.append(lambda r=rbs, f=_recips: r.append(f()))
                    pending.append(lambda r=rbs, f=_combine: f(r[0]))
            for fn in pending_evict:
                fn()
            pending_evict = []
            for fn in pending_pre:
                fn()
            pending_pre = []
            for fn in pending:
                fn()
            pending = []
            for fn in pending_next:
                fn()
            pending_next = []

            # ---- head finalize: per-partition (mean, var) over S leave the
            # device; the 64-way scalar reduction + rsqrt + affine happen on
            # the host during unsharding ----
            mv = pst.tile([65, 2], f32)
            nc.vector.bn_aggr(mv[:], st[:])
            nc.sync.dma_start(sgtb[h], mv[:])

    nc.compile()
    return nc


def _get_nc():
    if "nc" not in _CACHE:
        _CACHE["nc"] = _build_nc()
    return _CACHE["nc"]


def _host_prep(q, k, v, lq1, lq2, lk1, lk2, gamma, beta):
    """Build per-core input maps."""
    q = np.asarray(q, dtype=np.float32)
    k = np.asarray(k, dtype=np.float32)
    v = np.asarray(v, dtype=np.float32)
    lam = float(
        np.exp(np.float32(np.dot(lq1, lk1)))
        - np.exp(np.float32(np.dot(lq2, lk2)))
        + LAMBDA_INIT
    )
    g2 = (np.asarray(gamma, np.float32) * (1.0 - LAMBDA_INIT)).reshape(H, D)
    b2 = (np.asarray(beta, np.float32) * (1.0 - LAMBDA_INIT)).reshape(H, D)

    in_maps = []
    for c in range(N_CORES):
        heads = range(c * HPC, (c + 1) * HPC)
        qTa = np.empty((HPC, 128, S), np.float16)
        kTa = np.empty((HPC, 128, S), np.float16)
        vpa = np.empty((HPC, 2, 128, KB * 65), np.float16)
        gba = np.empty((HPC, 64, 2), np.float32)
        for i, hh in enumerate(heads):
            qTa[i] = q[0, hh].T.astype(np.float16)
            kTa[i] = k[0, hh].T.astype(np.float16)
            vh = v[0, hh]  # [S, 64]
            v1 = np.concatenate([np.ones((S, 1), np.float32), vh], axis=1)
            v2 = np.concatenate([np.ones((S, 1), np.float32), lam * vh], axis=1)
            # SBUF image: [partition(key within block), kblock*65 + col]
            vpa[i, 0] = (
                v1.reshape(KB, 128, 65).transpose(1, 0, 2).reshape(128, KB * 65)
            ).astype(np.float16)
            vpa[i, 1] = (
                v2.reshape(KB, 128, 65).transpose(1, 0, 2).reshape(128, KB * 65)
            ).astype(np.float16)
            gba[i, :, 0] = g2[hh]
            gba[i, :, 1] = b2[hh]
        in_maps.append({"qT": qTa, "kT": kTa, "vp": vpa, "gb": gba})
    return in_maps


def kernel(q, k, v, lq1, lq2, lk1, lk2, gamma, beta, _trace=False, _tmpdir=None):
    from concourse.bass_utils import run_bass_kernel_spmd

    nc = _get_nc()
    in_maps = _host_prep(q, k, v, lq1, lq2, lk1, lk2, gamma, beta)
    res = run_bass_kernel_spmd(
        nc,
        in_maps,
        core_ids=list(range(N_CORES)),
        trace=_trace,
        tmpdir=_tmpdir,
    )
    g2 = (np.asarray(gamma, np.float32) * (1.0 - LAMBDA_INIT)).reshape(H, D)
    b2 = (np.asarray(beta, np.float32) * (1.0 - LAMBDA_INIT)).reshape(H, D)
    out = np.empty((B, H, S, D), np.float32)
    for c in range(N_CORES):
        outT = res.results[c]["outT"]  # [HPC, 64, S] un-affined diff
        mvs = res.results[c]["sgtb"]   # [HPC, 65, 2] per-partition (mean, var)
        for i in range(HPC):
            hh = c * HPC + i
            mean_p = mvs[i, 1:65, 0].astype(np.float64)
            var_p = mvs[i, 1:65, 1].astype(np.float64)
            mu = mean_p.mean()
            var = (var_p + mean_p**2).mean() - mu * mu
            rstd = 1.0 / np.sqrt(var + EPS)
            sg = (rstd * g2[hh]).astype(np.float32)[:, None]
            tb = (b2[hh] - mu * rstd * g2[hh]).astype(np.float32)[:, None]
            out[0, hh] = (outT[i] * sg + tb).T
    if _trace:
        _CACHE["last_results"] = res
    return out


# revision 66
# speedup vs baseline: 1.1155x; 1.0359x over previous
"""Differential attention (two-softmax diff + GroupNorm) on 8 TRN2 cores.

Sharding: 16 heads / 8 cores = 2 heads per core (head-parallel, no
collectives). GroupNorm stats are per-(batch, head) so each core is fully
independent.

Device layout (host prepares everything):
  - Q, K per head are host-transposed to [128(d), 2048(s)] fp16: partitions
    0-63 hold half-1 (q1/k1), partitions 64-127 hold half-2. QK^T contracts
    over the partition dim, producing transposed score blocks S^T[key, query]
    in PSUM (fp32). The two 64-contraction halves auto-derive PE row-tile
    positions (0,0)/(64,0) and execute concurrently on the array.
  - V per head is prefixed with a ones column (V' = [1 | V], 65 cols, fp16)
    pre-arranged as [128(key-in-block), 16*65]: the PV matmul yields the
    softmax denominator on partition 0 and the numerator on partitions 1-64.
    lam is folded into half-2's V on the host.

ScalarE's exp over 2*S^2 scores/head is the bottleneck engine (~1.1 ns per
lane-element + ~260ns fixed per ACTIVATE). Levers:
  1. Bigger activation batches: scores accumulate in two rotating 3-bank
     PSUM tiles [128, 1536] so each ACTIVATE covers 3 slabs.
  2. ~25% of slab-groups are exp'd on the VectorE via a 2-instruction
     Schraudolph pipeline with cubic mantissa correction (max rel err
     ~7.7e-4, same class as the fp16 store quantization):
       i   = round_f32(s*A2 + B2)      stock tensor_scalar, f32->i32 convert
       w   = bitcast_f32(i)            = 2^(t+delta) * (1+f)/2^f
       m   = (i & 0x7FFFFF) | One.bits custom op: mantissa -> [1,2)
       e   = (((p'-m)m + q')m + r') * w   cubic corr, |c3| folded into B2
     The finisher is one 8-stage custom-DVE op (and, or, sub, mul, add,
     mul, add, mul) registered at import time.

Scheduling: per chunk, pass A emits all QK + exp (e production runs ahead),
pass B the serial PV accumulation; the next chunk's first two score groups
are emitted before this chunk's tail PVs (software-pipelined boundary).
Accumulators are evicted by ScalarE copies slotted into the next chunk's
act stream; the normalize (reciprocal_approx_fast on the den rows, GpSimd
partition broadcasts, multiply/subtract) is deferred behind the next
chunk's exp work on the DVE. bn stats stay on-device per chunk; the final
64-way scalar stat reduction, rsqrt and GroupNorm affine are applied on
the host during unsharding (outT carries the un-affined diff, sgtb the
per-partition (mean, var)).
"""

import math

import numpy as np

B, H, S, D = 1, 16, 2048, 64
N_CORES = 8
HPC = H // N_CORES  # heads per core
QC = 512            # query-chunk width
N_QC = S // QC
KB = S // 128       # key blocks of 128
LAMBDA_INIT = 0.8
EPS = 1e-5
SCALE = 1.0 / math.sqrt(D)
N_WARMUP_MM = 14

# cubic minimax fit of R(m) = 2^(m-1)/m on [1,2):  c3 m^3 + c2 m^2 + c1 m + c0
_C3 = -0.10246085749846692
_C2 = 0.69063801
_C1 = -1.35417106
_C0 = 1.76527539
PPRIME = -_C2 / _C3            # +6.7405058
QPRIME = -_C1 / _C3            # -13.216472
RPRIME = -_C0 / _C3            # +17.228778
DELTA = math.log2(-_C3)        # fold |c3| into the exponent bias
A2 = float(np.float32(math.log2(math.e) * SCALE * 2.0**23))
B2 = float(np.float32((127.0 + DELTA) * 2.0**23))

# slab-groups per 512-query chunk: 32 slabs of [128,512] scores -> 11 groups
GROUPS = [(i * 3, 3) for i in range(10)] + [(30, 2)]
# group indices handled by the VectorE exp pipeline (rest: ScalarE ACTIVATE);
# mid placement keeps the e-latency off both the rotation head and the PV
# chain tail; alternation balances the two engines at ~2.5 groups/chunk
DVE_GROUPS_EVEN = (2, 5, 10)
DVE_GROUPS_ODD = (3, 7, 10)
DVE_GROUPS_LAST = (3, 7)

_CACHE = {}


def _get_exp_op():
    """Register (once) and return the custom-DVE exp-finisher op."""
    if "expop" in _CACHE:
        return _CACHE["expop"]
    from concourse import dve_ops
    from concourse.dve_spec import (
        AluOp,
        Bin,
        C0,
        C1,
        C2,
        C3,
        One,
        Spec,
        Src0,
        _spill_c3_to_src1,
        lower,
    )
    from concourse.dve_uop import DveOpSpec

    for existing in dve_ops.OPS:
        if existing.name == "ANT_EXP2_FINISH":
            _CACHE["expop"] = existing
            return existing

    mm = Bin(AluOp.BITWISE_AND, Src0, C0)
    mo = Bin(AluOp.BITWISE_OR, mm, One)
    t5 = ((C1 - mo) * mo + C2) * mo + C3
    body = _spill_c3_to_src1(t5 * Src0)

    def _ref(in0, in1, s0, s1, imm2):
        bits = np.asarray(in0, np.float32).view(np.int32)
        s0i = np.asarray(s0).view(np.int32) if isinstance(s0, np.ndarray) else np.int32(s0)
        m = ((bits & s0i) | np.int32(0x3F800000)).view(np.float32)
        t = ((np.float32(s1) - m) * m + np.float32(imm2)) * m + np.asarray(
            in1, np.float32
        )
        return t * np.asarray(in0, np.float32)

    spec = Spec(body=body, reference=_ref)
    op = dve_ops.DveOp("ANT_EXP2_FINISH", spec, subdim=False, uops_sha={})
    dve_ops.OPS.append(op)
    dve_ops._SUB_OPCODE_FOR_NAME[op.name] = dve_ops._CUSTOM_DVE_ROW_BASE + len(
        dve_ops.OPS
    ) - 1
    dve_ops.CUSTOM_DVE_SPECS[op.name] = spec
    for ver in ("v3", "v4"):
        tmp = DveOpSpec(
            name=op.name,
            opcode=dve_ops.get_dve_sub_opcode(op.name),
            uops=lower(spec, ver=ver),
            rd1_en=True,
        )
        op.uops_sha[ver] = tmp.sha(ver)
    _CACHE["expop"] = op
    return op


def _build_nc():
    from contextlib import ExitStack

    import concourse.bacc as bacc
    import concourse.bass as bass
    import concourse.tile as tile
    from concourse import bass_isa, mybir

    f32 = mybir.dt.float32
    f16 = mybir.dt.float16
    i32 = mybir.dt.int32
    AF = mybir.ActivationFunctionType
    OP = mybir.AluOpType
    ts = bass.ts

    expop = _get_exp_op()

    nc = bacc.Bacc("TRN2", target_bir_lowering=False, debug=False)

    qT = nc.dram_tensor("qT", [HPC, 128, S], f16, kind="ExternalInput").ap()
    kT = nc.dram_tensor("kT", [HPC, 128, S], f16, kind="ExternalInput").ap()
    vp = nc.dram_tensor("vp", [HPC, 2, 128, KB * 65], f16, kind="ExternalInput").ap()
    gb = nc.dram_tensor("gb", [HPC, 64, 2], f32, kind="ExternalInput").ap()
    outT = nc.dram_tensor("outT", [HPC, 64, S], f32, kind="ExternalOutput").ap()
    sgtb = nc.dram_tensor("sgtb", [HPC, 65, 2], f32, kind="ExternalOutput").ap()

    with tile.TileContext(nc) as tc, ExitStack() as ctx:
        pq = ctx.enter_context(tc.tile_pool(name="pq", bufs=2))
        pk = ctx.enter_context(tc.tile_pool(name="pk", bufs=2))
        pv = ctx.enter_context(tc.tile_pool(name="pv", bufs=2))
        pe = ctx.enter_context(tc.tile_pool(name="pe", bufs=13))
        pw = ctx.enter_context(tc.tile_pool(name="pw", bufs=1))
        pep = ctx.enter_context(tc.tile_pool(name="pep", bufs=2))
        psa = ctx.enter_context(tc.tile_pool(name="psa", bufs=2))
        pout = ctx.enter_context(tc.tile_pool(name="pout", bufs=2))
        pst = ctx.enter_context(tc.tile_pool(name="pst", bufs=2))
        psingle = ctx.enter_context(tc.tile_pool(name="psingle", bufs=1))
        psc = ctx.enter_context(tc.tile_pool(name="psc", bufs=1, space="PSUM"))
        pacc = ctx.enter_context(tc.tile_pool(name="pacc", bufs=1, space="PSUM"))

        def emit_loads(h):
            """DMA the head's inputs; split so the first matmuls start early."""
            ksh = []
            for j in range(2):
                ks_t = pk.tile([128, S // 2], f16, tag=f"ks{j}", name=f"ks{j}")
                ksh.append(ks_t)
            qsh = []
            for j in range(N_QC):
                qs_t = pq.tile([128, QC], f16, tag=f"qs{j}", name=f"qs{j}")
                qsh.append(qs_t)
            nc.sync.dma_start(ksh[0][:], kT[h, :, 0 : S // 2])
            nc.sync.dma_start(qsh[0][:], qT[h, :, 0:QC])
            nc.sync.dma_start(ksh[1][:], kT[h, :, S // 2 : S])
            for j in range(1, N_QC):
                nc.sync.dma_start(qsh[j][:], qT[h, :, j * QC : (j + 1) * QC])
            vsh = []
            for half in range(2):
                row = []
                for j in range(2):
                    t = pv.tile(
                        [128, KB * 65 // 2], f16, tag=f"v{half}{j}", name=f"v{half}{j}"
                    )
                    nc.sync.dma_start(
                        t[:],
                        vp[h, half, :, j * (KB * 65 // 2) : (j + 1) * (KB * 65 // 2)],
                    )
                    row.append(t)
                vsh.append(row)
            return ksh, qsh, vsh

        # PE warm-up: tiny back-to-back matmuls flip the HAM clock gate to
        # 8/8 while the first head's DMAs are in flight.
        wu_w = psingle.tile([128, 128], f16)
        nc.vector.memset(wu_w, 0.0)
        wu_ps = psc.tile([128, 3 * QC], f32, tag="sc0")
        for _ in range(N_WARMUP_MM):
            nc.tensor.matmul(
                wu_ps[:, 0:128], lhsT=wu_w[:], rhs=wu_w[:], start=True, stop=True
            )

        loads = emit_loads(0)

        mask_t = psingle.tile([128, 1], i32)
        nc.vector.memset(mask_t, 0x007FFFFF)
        rprime_t = psingle.tile([128, 1], f32)
        nc.vector.memset(rprime_t, RPRIME)

        for h in range(HPC):
            ksh, qsh, vsh = loads
            if h + 1 < HPC:
                # prefetch the next head's inputs behind this head's compute
                loads = emit_loads(h + 1)


            outc = pout.tile([65, S], f32)
            st = pst.tile([65, N_QC, 6], f32)

            e_tiles = {}  # (qc, g) -> e tile
            accs = {}     # qc -> (a1, a2)

            def emit_A(qc, g):
                """Scores + exp for group g of chunk qc."""
                gs, gl = GROUPS[g]
                L = gl * QC
                last = h == HPC - 1 and qc == N_QC - 1
                dve_groups = (
                    DVE_GROUPS_LAST
                    if last
                    else (DVE_GROUPS_EVEN if qc % 2 == 0 else DVE_GROUPS_ODD)
                )
                sc = psc.tile([128, 3 * QC], f32, tag=f"sc{g % 2}")
                for i in range(gs, gs + gl):
                    kb, half = divmod(i, 2)
                    col = (i - gs) * QC
                    ksk = ksh[kb // 8][:, ts(kb % 8, 128)]
                    nc.tensor.matmul(
                        sc[:, col : col + QC],
                        lhsT=ksk[64 * half : 64 * (half + 1), :],
                        rhs=qsh[qc][64 * half : 64 * (half + 1), :],
                        start=True,
                        stop=True,
                    )
                e = pe.tile([128, 3 * QC], f16, tag="e")
                e_tiles[(qc, g)] = e
                if g in dve_groups:
                    w32 = pw.tile([128, 3 * QC], i32, tag=f"w{g % 2}")
                    nc.vector.tensor_scalar(
                        out=w32[:, 0:L],
                        in0=sc[:, 0:L],
                        scalar1=A2,
                        scalar2=B2,
                        op0=OP.mult,
                        op1=OP.add,
                    )
                    nc.vector._custom_dve(
                        expop,
                        out=e[:, 0:L],
                        in0=w32[:, 0:L].bitcast(f32),
                        in1=rprime_t[:],
                        s0=mask_t[:].bitcast(f32),
                        s1=PPRIME,
                        imm2=QPRIME,
                    )
                else:
                    nc.scalar.activation(e[:, 0:L], sc[:, 0:L], AF.Exp, scale=SCALE)

            def emit_B(qc, g):
                """PV accumulation for group g of chunk qc."""
                gs, gl = GROUPS[g]
                a1, a2 = accs[qc]
                for i in range(gs, gs + gl):
                    kb, half = divmod(i, 2)
                    col = (i - gs) * QC
                    acc = a1 if half == 0 else a2
                    vk = vsh[half][kb // 8][:, ts(kb % 8, 65)]
                    nc.tensor.matmul(
                        acc[:],
                        lhsT=vk,
                        rhs=e_tiles.pop((qc, g))[:, col : col + QC]
                        if i == gs + gl - 1
                        else e_tiles[(qc, g)][:, col : col + QC],
                        start=(kb == 0),
                        stop=(kb == KB - 1),
                    )

            pending_evict = []
            pending_pre = []
            pending = []
            pending_next = []
            for qc in range(N_QC):
                last = h == HPC - 1 and qc == N_QC - 1
                if qc == 0:
                    emit_A(0, 0)
                    emit_A(0, 1)
                for g in range(2, len(GROUPS)):
                    if g == 2:
                        # previous chunk's accumulator evictions slot in here
                        # so they never head-of-line block this chunk's first
                        # activations on the ScalarE queue; the reciprocals
                        # follow and fill the DVE's early idle window
                        for fn in pending_evict:
                            fn()
                        pending_evict = []
                        for fn in pending_pre:
                            fn()
                        pending_pre = []
                    emit_A(qc, g)
                # previous chunk's deferred combine sits behind this chunk's
                # exp stream on the DVE
                for fn in pending:
                    fn()
                pending = pending_next
                pending_next = []

                acc1 = pacc.tile([65, QC], f32, tag="a1")
                acc2 = pacc.tile([65, QC], f32, tag="a2")
                accs[qc] = (acc1, acc2)
                for g in range(0, 9):
                    emit_B(qc, g)
                # software-pipeline the chunk boundary: the next chunk's first
                # score groups go ahead of this chunk's tail PVs in the PE
                # stream so the activation rotation never stalls on them
                if qc + 1 < N_QC:
                    emit_A(qc + 1, 0)
                    emit_A(qc + 1, 1)
                for g in range(9, len(GROUPS)):
                    emit_B(qc, g)

                a1, a2 = accs.pop(qc)
                if last:
                    # banks are free after this — normalize straight out of
                    # PSUM, halves pipelined, to shorten the exposed tail
                    rb1r = pst.tile([1, QC], f32, tag="rb1r")
                    nc.vector.reciprocal_approx_fast(rb1r[:], a1[0:1, :])
                    rb1 = pep.tile([65, QC], f32, tag="rb1")
                    nc.gpsimd.partition_broadcast(rb1[:], rb1r[:], channels=65)
                    rb2r = pst.tile([1, QC], f32, tag="rb2r")
                    nc.vector.reciprocal_approx_fast(rb2r[:], a2[0:1, :])
                    rb2 = pep.tile([65, QC], f32, tag="rb2")
                    nc.gpsimd.partition_broadcast(rb2[:], rb2r[:], channels=65)
                    t1 = pep.tile([65, QC], f32, tag="lt1")
                    nc.vector.tensor_mul(t1[:], a1[:], rb1[:])
                    t2 = pep.tile([65, QC], f32, tag="lt2")
                    nc.vector.tensor_mul(t2[:], a2[:], rb2[:])
                    nc.vector.tensor_sub(outc[:, ts(qc, QC)], t1[:], t2[:])
                    nc.vector.bn_stats(st[:, qc, :], outc[:, ts(qc, QC)])
                    nc.sync.dma_start(outT[h, :, ts(qc, QC)], outc[1:65, ts(qc, QC)])
                else:
                    # reciprocals inline, straight from the PSUM den rows:
                    # they are ready at the chunk seam (no dependency on the
                    # eviction copies) so they fill the DVE's seam idle
                    # instead of head-of-line blocking the next chunk's exps
                    rb1r = pst.tile([1, QC], f32, tag="rb1r")
                    nc.vector.reciprocal_approx_fast(rb1r[:], a1[0:1, :])
                    rb1 = pep.tile([65, QC], f32, tag="rb1")
                    nc.gpsimd.partition_broadcast(rb1[:], rb1r[:], channels=65)
                    rb2r = pst.tile([1, QC], f32, tag="rb2r")
                    nc.vector.reciprocal_approx_fast(rb2r[:], a2[0:1, :])
                    rb2 = pep.tile([65, QC], f32, tag="rb2")
                    nc.gpsimd.partition_broadcast(rb2[:], rb2r[:], channels=65)
                    sa1 = psa.tile([65, QC], f32, tag=f"sa1_{qc % 2}")
                    sa2 = psa.tile([65, QC], f32, tag=f"sa2_{qc % 2}")

                    def _evict(a1=a1, a2=a2, sa1=sa1, sa2=sa2):
                        nc.scalar.copy(sa1[:], a1[:])
                        nc.scalar.copy(sa2[:], a2[:])

                    def _combine(qc=qc, sa1=sa1, sa2=sa2, rb1=rb1, rb2=rb2, h=h):
                        t1 = pep.tile([65, QC], f32, tag=f"t1_{qc % 2}")
                        nc.vector.tensor_mul(t1[:], sa1[:], rb1[:])
                        t2 = pep.tile([65, QC], f32, tag=f"t2_{qc % 2}")
                        nc.vector.tensor_mul(t2[:], sa2[:], rb2[:])
                        nc.vector.tensor_sub(outc[:, ts(qc, QC)], t1[:], t2[:])
                        nc.vector.bn_stats(st[:, qc, :], outc[:, ts(qc, QC)])
                        # un-affined diff streams out now; the host applies
                        # the per-head GroupNorm affine during unsharding
                        nc.sync.dma_start(
                            outT[h, :, ts(qc, QC)], outc[1:65, ts(qc, QC)]
                        )

                    pending_evict.append(_evict)
                    pending.append(_combine)
            for fn in pending_evict:
                fn()
            pending_evict = []
            for fn in pending_pre:
                fn()
            pending_pre = []
            for fn in pending:
                fn()
            pending = []
            for fn in pending_next:
                fn()
            pending_next = []

            # ---- head finalize: per-partition (mean, var) over S leave the
            # device; the 64-way scalar reduction + rsqrt + affine happen on
            # the host during unsharding ----
            mv = pst.tile([65, 2], f32)
            nc.vector.bn_aggr(mv[:], st[:])
            nc.sync.dma_start(sgtb[h], mv[:])

    nc.compile()
    return nc


def _get_nc():
    if "nc" not in _CACHE:
        _CACHE["nc"] = _build_nc()
    return _CACHE["nc"]


def _host_prep(q, k, v, lq1, lq2, lk1, lk2, gamma, beta):
    """Build per-core input maps."""
    q = np.asarray(q, dtype=np.float32)
    k = np.asarray(k, dtype=np.float32)
    v = np.asarray(v, dtype=np.float32)
    lam = float(
        np.exp(np.float32(np.dot(lq1, lk1)))
        - np.exp(np.float32(np.dot(lq2, lk2)))
        + LAMBDA_INIT
    )
    g2 = (np.asarray(gamma, np.float32) * (1.0 - LAMBDA_INIT)).reshape(H, D)
    b2 = (np.asarray(beta, np.float32) * (1.0 - LAMBDA_INIT)).reshape(H, D)

    in_maps = []
    for c in range(N_CORES):
        heads = range(c * HPC, (c + 1) * HPC)
        qTa = np.empty((HPC, 128, S), np.float16)
        kTa = np.empty((HPC, 128, S), np.float16)
        vpa = np.empty((HPC, 2, 128, KB * 65), np.float16)
        gba = np.empty((HPC, 64, 2), np.float32)
        for i, hh in enumerate(heads):
            qTa[i] = q[0, hh].T.astype(np.float16)
            kTa[i] = k[0, hh].T.astype(np.float16)
            vh = v[0, hh]  # [S, 64]
            v1 = np.concatenate([np.ones((S, 1), np.float32), vh], axis=1)
            v2 = np.concatenate([np.ones((S, 1), np.float32), lam * vh], axis=1)
            # SBUF image: [partition(key within block), kblock*65 + col]
            vpa[i, 0] = (
                v1.reshape(KB, 128, 65).transpose(1, 0, 2).reshape(128, KB * 65)
            ).astype(np.float16)
            vpa[i, 1] = (
                v2.reshape(KB, 128, 65).transpose(1, 0, 2).reshape(128, KB * 65)
            ).astype(np.float16)
            gba[i, :, 0] = g2[hh]
            gba[i, :, 1] = b2[hh]
        in_maps.append({"qT": qTa, "kT": kTa, "vp": vpa, "gb": gba})
    return in_maps


def kernel(q, k, v, lq1, lq2, lk1, lk2, gamma, beta, _trace=False, _tmpdir=None):
    from concourse.bass_utils import run_bass_kernel_spmd

    nc = _get_nc()
    in_maps = _host_prep(q, k, v, lq1, lq2, lk1, lk2, gamma, beta)
    res = run_bass_kernel_spmd(
        nc,
        in_maps,
        core_ids=list(range(N_CORES)),
        trace=_trace,
        tmpdir=_tmpdir,
    )
    g2 = (np.asarray(gamma, np.float32) * (1.0 - LAMBDA_INIT)).reshape(H, D)
    b2 = (np.asarray(beta, np.float32) * (1.0 - LAMBDA_INIT)).reshape(H, D)
    out = np.empty((B, H, S, D), np.float32)
    for c in range(N_CORES):
        outT = res.results[c]["outT"]  # [HPC, 64, S] un-affined diff
        mvs = res.results[c]["sgtb"]   # [HPC, 65, 2] per-partition (mean, var)
        for i in range(HPC):
            hh = c * HPC + i
            mean_p = mvs[i, 1:65, 0].astype(np.float64)
            var_p = mvs[i, 1:65, 1].astype(np.float64)
            mu = mean_p.mean()
            var = (var_p + mean_p**2).mean() - mu * mu
            rstd = 1.0 / np.sqrt(var + EPS)
            sg = (rstd * g2[hh]).astype(np.float32)[:, None]
            tb = (b2[hh] - mu * rstd * g2[hh]).astype(np.float32)[:, None]
            out[0, hh] = (outT[i] * sg + tb).T
    if _trace:
        _CACHE["last_results"] = res
    return out
